# revision 13
# baseline (speedup 1.0000x reference)
"""Trainium2 Bass kernel for an LSTM decoder with additive attention + large
vocab projection (nn_DecoderWithAttention).

Strategy: 8-way data parallel over batch (8 samples per core), zero
collectives. Recurrent state h is kept feature-major [D, B] and scaled by 2
(h' = 2h) so every sigmoid can be computed as sigma(x) = (1 + tanh(x/2)) / 2
on the ACT engine -- keeping all scalar-engine ops inside the single
"exp_and_others" activation table set. The compensating 0.5 factors are
folded into W_d/W_beta/W_hh/W_fc/W_ih2 on the host (exact power-of-two
scale).

v2 changes vs v1 (1.90 ms):
  - fc phase split into two bf16 passes over H (t=1..16 and t=17..29); the
    first pass is emitted interleaved into steps 16..28 so its matmuls/DMAs
    hide in the recurrent loop's idle time. W_fc and the fc copy of h are
    bf16 (~0.4% rounding on logits only).
  - PSUM rebudgeted (8 banks: small 1 / scores 4 / ctx+fc 1 / beta+gates 2)
    so the beta matmul and ~half of the gates h-part matmuls prefill at step
    start, overlapping the attention phase.
  - Attention tensors (enc_projT / decT / sT / eT / wf) in fp16: halves
    SBUF + lets the scores matmul stream fp16.
  - alpha scatter into the block-diagonal tile is one strided tensor_copy.
  - dec_proj uses bf16 Wd x bf16 h' (1 cyc/row vs fp32's 4).

Per-core per-step dataflow (s = 0..28):
  dec_projT [A,B]  = (0.5 Wd)^T @ h'          (PE, bf16)
  beta/gates h-part matmuls prefill           (PE, during attention)
  eT[A,(B,P)]      = tanh(enc_projT + dec_projT bcast)   (DVE add, ACT tanh)
  scores[1,(B,P)]  = wf^T @ eT                (PE, fp16 streaming)
  alpha            = exp(scores)/sum          (ACT exp + fused accum)
  ctx[B,ENC]       = alpha @ feats            (PE, block-diag 16-K accum)
  ctx2             = (1 + tanh(zb/2)) * ctx   (= 2 sigmoid(zb) ctx)
  gates[B,4D]      = emb_pre[s] + ctx2@(W_ih2/2) + h'@(W_hh/2)
  LSTM cell in tanh form; h' = (1+tanh(o/2)) * tanh(c2)
Output row t=0 stays zero (buffer pre-zeroed + explicit zero DMA).
"""

import os
import sys

for _p in ("/opt/trn_rl_repo", os.path.expanduser("~/.axon_site/_ro/trn_rl_repo")):
    if os.path.isdir(_p) and _p not in sys.path:
        sys.path.insert(0, _p)

import numpy as np

import concourse.bass as bass
import concourse.tile as tile
from concourse import bacc, mybir
from concourse.bass_utils import run_bass_kernel_spmd
from concourse.masks import make_identity

F32 = mybir.dt.float32
F32R = mybir.dt.float32r
F16 = mybir.dt.float16
BF16 = mybir.dt.bfloat16

B, P, T = 64, 196, 30
E, D, A, ENC, V = 512, 512, 512, 512, 30000
NCORES = 8
BL = B // NCORES          # 8 samples per core
S = T - 1                 # 29 recurrent steps
BP = BL * P               # 1568
KC = 4                    # 128-row chunks per 512 feature dim
VCHUNK = 512
V_CHUNKS = [(i * VCHUNK, min(VCHUNK, V - i * VCHUNK))
            for i in range((V + VCHUNK - 1) // VCHUNK)]
# fc pass 1 (t=1..13, 104 rows) is interleaved into steps FC_S0..28.
FC_S0 = 13
FC_PER_STEP = 4

Tanh = mybir.ActivationFunctionType.Tanh
Exp = mybir.ActivationFunctionType.Exp
Ident = mybir.ActivationFunctionType.Identity
Add = mybir.AluOpType.add
Mult = mybir.AluOpType.mult


def r(ap):
    return ap.bitcast(F32R)


def build_program(with_biases=False):
    nc = bacc.Bacc(
        "TRN2",
        target_bir_lowering=False,
        debug=False,
        enable_asserts=False,
        num_devices=NCORES,
    )

    def din(name, shape, dt=F32):
        return nc.dram_tensor(name, list(shape), dt, kind="ExternalInput").ap()

    featsT_d = din("featsT", (128, KC, BP), BF16)          # [p,c,b*196+q] = feats[b,q,128c+p]
    feats16_d = din("feats16", (128, 2 * BL, ENC), BF16)   # (b,p) rows, P padded to 256
    pooledT_d = din("pooledT", (128, KC, BL), F32R)
    embT_d = din("embT", (128, KC, 32, BL), BF16)          # [p,c,t,b], t<29 used
    Wd16_d = din("Wd16", (128, KC, A), BF16)               # 0.5*Wd_att rows, bf16
    wf_d = din("wf16", (128, KC, 1), F16)
    Wih2_d = din("Wih2", (128, KC, 4 * D), BF16)           # 0.5*W_ih[512:] rows
    Whh_d = din("Whh", (128, KC, 4 * D), F32R)             # 0.5*W_hh rows
    WihE_d = din("WihE", (128, KC, 4 * D), BF16)           # W_ih[:512] rows
    Wbeta_d = din("Wbeta", (128, KC, ENC), BF16)           # 0.5*W_beta rows
    Winih_d = din("Winih", (128, KC, D), F32R)
    Winic_d = din("Winic", (128, KC, D), F32R)
    We_d = din("We", (128, KC, A), BF16)                   # We_att rows
    Wfc16_d = din("Wfc16", (128, KC, V), BF16)             # 0.5*W_fc rows, bf16
    bihh_d = din("bihh", (1, 4 * D), F32R)                 # b_ih + b_hh
    binih_d = din("binih", (1, D), F32R)
    binic_d = din("binic", (1, D), F32R)
    bd_d = din("bd_til", (128, KC))                        # bd_att as [p, c]
    be_d = din("be_til", (128, KC))
    ones_d = din("ones128", (1, 128), F32R)
    if with_biases:
        bbeta_d = din("bbeta", (1, ENC), F32R)

    out_d = nc.dram_tensor("out_logits", [BL, T, V], F32, kind="ExternalOutput").ap()
    outT = out_d.rearrange("b t v -> t b v")

    with tile.TileContext(nc) as tc:
        with tc.tile_pool(name="const", bufs=1) as const:
            wd_sb = const.tile([128, KC, A], BF16)
            wf_sb = const.tile([128, KC, 1], F16)
            wih2_sb = const.tile([128, KC, 4 * D], BF16)
            whh_sb = const.tile([128, KC, 4 * D], F32R)
            wbeta_sb = const.tile([128, KC, ENC], BF16)
            enc_projT = const.tile([128, KC, BP], F16)
            emb_pre = const.tile([128, 2, 4 * D], F32R)
            H_all = const.tile([128, KC, T, BL], F32R)   # slot t: h' after t steps
            H16 = const.tile([128, KC, T, BL], BF16)     # bf16 copy for fc + dec
            bd_sb = const.tile([128, KC], F32)
            be_sb = const.tile([128, KC], F32)
            ident = const.tile([BL, BL], F32)
            ident128 = const.tile([128, 128], F32)
            idsel = const.tile([128, 128], F32R)
            c_state = [const.tile([BL, D], F32, tag=f"cstate{i}", name=f"c_state{i}")
                       for i in range(2)]
            if with_biases:
                ones_lp = const.tile([1, 128], F32R)
                bbeta_sb = const.tile([1, ENC], F32R)
                nc.sync.dma_start(ones_lp[:], ones_d)
                nc.sync.dma_start(bbeta_sb[:], bbeta_d)

            nc.sync.dma_start(wd_sb[:], Wd16_d)
            nc.sync.dma_start(wf_sb[:], wf_d)
            nc.sync.dma_start(wih2_sb[:], Wih2_d)
            nc.sync.dma_start(whh_sb[:], Whh_d)
            nc.sync.dma_start(wbeta_sb[:], Wbeta_d)
            nc.sync.dma_start(bd_sb[:], bd_d)
            nc.sync.dma_start(be_sb[:], be_d)
            make_identity(nc, ident[:])
            make_identity(nc, ident128[:])
            nc.vector.tensor_copy(idsel[:], ident128[:])

            # ---------------- setup phase ----------------
            with tc.tile_pool(name="setup", bufs=1) as setup, \
                 tc.tile_pool(name="setup2", bufs=2) as setup2, \
                 tc.tile_pool(name="setup_ps", bufs=2, space="PSUM") as setup_ps:

                pooledT_sb = setup.tile([128, KC, BL], F32R)
                ones_sb = setup.tile([1, 128], F32R)
                bihh_sb = setup.tile([1, 4 * D], F32R)
                binih_sb = setup.tile([1, D], F32R)
                binic_sb = setup.tile([1, D], F32R)
                winih_sb = setup.tile([128, KC, D], F32R)
                winic_sb = setup.tile([128, KC, D], F32R)
                nc.sync.dma_start(pooledT_sb[:], pooledT_d)
                nc.sync.dma_start(ones_sb[:], ones_d)
                nc.sync.dma_start(bihh_sb[:], bihh_d)
                nc.sync.dma_start(binih_sb[:], binih_d)
                nc.sync.dma_start(binic_sb[:], binic_d)
                nc.sync.dma_start(winih_sb[:], Winih_d)
                nc.sync.dma_start(winic_sb[:], Winic_d)

                # h0/c0 (B-major): lhsT = pooledT chunks, rhs = W_init rows
                for which in range(2):
                    w_sb = winih_sb if which == 0 else winic_sb
                    b_row = binih_sb if which == 0 else binic_sb
                    ps = setup_ps.tile([BL, D], F32, tag="init_ps")
                    for kc in range(KC):
                        nc.tensor.matmul(ps[:], pooledT_sb[:, kc, :],
                                         w_sb[:, kc, :], start=(kc == 0), stop=False)
                    nc.tensor.matmul(ps[:], ones_sb[0:1, 0:BL], b_row[0:1, :],
                                     start=False, stop=True)
                    if which == 0:
                        h0 = setup.tile([BL, D], F32)
                        nc.scalar.activation(h0[:], ps[:], Tanh)
                        h0x2 = setup.tile([BL, D], F32)
                        nc.vector.tensor_scalar_mul(h0x2[:], h0[:], 2.0)
                        trps = setup_ps.tile([128, KC, BL], F32, tag="tr_ps")
                        for c in range(KC):
                            nc.tensor.transpose(trps[:, c, :],
                                                h0x2[:, c * 128:(c + 1) * 128],
                                                ident[:])
                        nc.vector.tensor_copy(H_all[:, :, 0, :], trps[:])
                        nc.scalar.activation(H16[:, :, 0, :], trps[:], Ident)
                    else:
                        nc.scalar.activation(c_state[0][:], ps[:], Tanh)

                # enc_projT = We^T @ featsT + be  (A-major, fp16 out)
                we_sb = setup.tile([128, KC, A], BF16)
                nc.sync.dma_start(we_sb[:], We_d)
                for n in range(KC):
                    nsl = bass.ts(n, BP // KC)  # 392 cols
                    ft_stage = setup2.tile([128, KC, BP // KC], BF16, tag="ftst")
                    nc.sync.dma_start(ft_stage[:], featsT_d[:, :, nsl])
                    for c in range(KC):
                        ps = setup_ps.tile([128, BP // KC], F32, tag="enc_ps")
                        for kc in range(KC):
                            nc.tensor.matmul(ps[:], we_sb[:, kc, bass.ts(c, 128)],
                                             ft_stage[:, kc, :],
                                             start=(kc == 0), stop=(kc == KC - 1))
                        nc.scalar.activation(enc_projT[:, c, nsl], ps[:], Ident,
                                             bias=be_sb[:, c:c + 1])

                # emb_pre = embT^T @ W_ih[:512] + (b_ih + b_hh), rows (t, b)
                nc.vector.memset(emb_pre[:].bitcast(F32), 0.0)
                embT_sb = setup.tile([128, KC, 32, BL], BF16)
                nc.sync.dma_start(embT_sb[:], embT_d)
                for n in range(4):
                    nsl = bass.ts(n, 512)
                    wst = setup2.tile([128, KC, 512], BF16, tag="wihE")
                    nc.sync.dma_start(wst[:], WihE_d[:, :, nsl])
                    for mt, (t0, nt) in enumerate([(0, 16), (16, 13)]):
                        rows = nt * BL
                        ps = setup_ps.tile([128, 512], F32, tag="emb_ps")
                        for kc in range(KC):
                            nc.tensor.matmul(ps[0:rows, :],
                                             embT_sb[:, kc, t0:t0 + nt, :],
                                             wst[:, kc, :], start=(kc == 0),
                                             stop=False)
                        nc.tensor.matmul(ps[0:rows, :], ones_sb[0:1, 0:rows],
                                         bihh_sb[0:1, nsl], start=False, stop=True)
                        nc.vector.tensor_copy(emb_pre[0:rows, mt, nsl], ps[0:rows, :])

            # ------------- recurrent loop + interleaved fc pass 1 -------------
            with tc.tile_pool(name="fcw1", bufs=3) as fcw1, \
                 tc.tile_pool(name="fcw2", bufs=6) as fcw2, \
                 tc.tile_pool(name="fco", bufs=3) as fco, \
                 tc.tile_pool(name="ps_cf", bufs=2, space="PSUM") as ps_cf:

                def fc_chunk(ci, t0, nt, wpool, wtag):
                    v0, vn = V_CHUNKS[ci]
                    rows = nt * BL
                    wst = wpool.tile([128, KC, VCHUNK], BF16, tag=wtag,
                                     name=f"fw{wtag}{ci}")
                    nc.sync.dma_start(wst[:, :, 0:vn], Wfc16_d[:, :, v0:v0 + vn])
                    ps = ps_cf.tile([128, VCHUNK], F32, tag="cf",
                                    name=f"psfc{t0}_{ci}")
                    for kc in range(KC):
                        nc.tensor.matmul(ps[0:rows, 0:vn],
                                         H16[:, kc, t0:t0 + nt, :],
                                         wst[:, kc, 0:vn],
                                         start=(kc == 0), stop=(kc == KC - 1))
                    ost = fco.tile([128, VCHUNK], F32, tag="fco",
                                   name=f"fo{t0}_{ci}")
                    nc.vector.tensor_copy(ost[0:rows, 0:vn], ps[0:rows, 0:vn])
                    nc.sync.dma_start(outT[t0:t0 + nt, :, v0:v0 + vn],
                                      ost[0:rows, 0:vn])

                # output row t=0 is defined to be zeros
                zt = fco.tile([1, 400], F32, tag="zt")
                nc.vector.memset(zt[:], 0.0)
                zsrc = zt[0:1, 0:400]
                zsrc = bass.AP(tensor=zsrc.tensor, offset=zsrc.offset,
                               ap=[list(zsrc.ap[0]), [0, BL * V // 400], [1, 400]])
                nc.sync.dma_start(out_d[:, 0, :], zsrc)

                with tc.tile_pool(name="lper", bufs=1) as lper, \
                     tc.tile_pool(name="sT", bufs=2) as sTp, \
                     tc.tile_pool(name="eT", bufs=4) as eTp, \
                     tc.tile_pool(name="scr", bufs=2) as scrp, \
                     tc.tile_pool(name="t2k", bufs=4) as t2k, \
                     tc.tile_pool(name="tiny", bufs=3) as tinyp, \
                     tc.tile_pool(name="sm", bufs=2) as smp, \
                     tc.tile_pool(name="ps_small", bufs=1, space="PSUM") as ps_small, \
                     tc.tile_pool(name="ps_sc", bufs=1, space="PSUM") as ps_sc_pool, \
                     tc.tile_pool(name="ps_bg", bufs=3, space="PSUM") as ps_bg:

                    feats16 = lper.tile([128, 2 * BL, ENC], BF16)
                    nc.sync.dma_start(feats16[:], feats16_d)
                    alphaD = lper.tile([128, 2 * BL, BL], BF16)
                    nc.vector.memset(alphaD[:], 0.0)
                    scores_buf = lper.tile([BL, 256], F32)
                    nc.vector.memset(scores_buf[:, P:256], 0.0)
                    # strided view of alphaD for the one-copy block-diag
                    # scatter: dst[b, j] = alphaD[:, 2b+j, b]
                    aD0 = alphaD[:]
                    alphaD_diag = bass.AP(
                        tensor=aD0.tensor, offset=aD0.offset,
                        ap=[list(aD0.ap[0]), [2 * BL + 1, BL], [BL, 2]])

                    fc_next = 0
                    for s in range(S):
                        hT = H_all[:, :, s, :]
                        c_prev = c_state[s % 2]
                        c_next = c_state[(s + 1) % 2]

                        # 1) dec_projT [128, KC, BL] (bf16 Wd x bf16 h')
                        ps_dec = ps_small.tile([128, KC, BL], F32, tag="small",
                                               name=f"psdec{s}")
                        for m in range(KC):
                            for kc in range(KC):
                                nc.tensor.matmul(ps_dec[:, m, :],
                                                 wd_sb[:, kc, bass.ts(m, 128)],
                                                 H16[:, kc, s, :],
                                                 start=(kc == 0),
                                                 stop=(kc == KC - 1))
                        decT = tinyp.tile([128, KC, BL], F16, tag="tiny",
                                          name=f"decT{s}")
                        for c in range(KC):
                            nc.scalar.activation(decT[:, c, :], ps_dec[:, c, :],
                                                 Ident, bias=bd_sb[:, c:c + 1])

                        # beta gate preact (h-only): prefills during attention
                        ps_b = ps_bg.tile([BL, ENC], F32, tag="bg",
                                          name=f"psb{s}")
                        for kc in range(KC):
                            nc.tensor.matmul(ps_b[:], H16[:, kc, s, :],
                                             wbeta_sb[:, kc, :],
                                             start=(kc == 0),
                                             stop=(not with_biases and kc == KC - 1))
                        if with_biases:
                            nc.tensor.matmul(ps_b[:], ones_lp[0:1, 0:BL],
                                             bbeta_sb[0:1, :], start=False, stop=True)
                        taub = t2k.tile([BL, ENC], F32, tag="t2k", name=f"taub{s}")
                        nc.scalar.activation(taub[:], ps_b[:], Tanh, scale=0.5)

                        # gates: h-part + emb selector prefill for n=0,1
                        erow = (s % 16) * BL
                        ps_g = []
                        for n in range(4):
                            ps_g.append(ps_bg.tile([BL, D], F32, tag="bg",
                                                   name=f"psg{s}_{n}"))
                        for n in range(2):
                            nsl = bass.ts(n, D)
                            for kc in range(KC):
                                nc.tensor.matmul(ps_g[n][:], hT[:, kc, :],
                                                 whh_sb[:, kc, nsl],
                                                 start=(kc == 0), stop=False)
                            nc.tensor.matmul(ps_g[n][:],
                                             idsel[:, erow:erow + BL],
                                             emb_pre[:, s // 16, nsl],
                                             start=False, stop=False)

                        # 2..5) e = tanh(enc_proj + dec_proj); scores = wf^T e
                        # last add chunk runs on GpSimd so DVE only does 3
                        eTs = []
                        for c in range(KC):
                            sT = sTp.tile([128, BL, P], F16, tag="sT",
                                          name=f"sT{s}_{c}")
                            eng = nc.gpsimd if c == KC - 1 else nc.vector
                            eng.tensor_tensor(
                                sT[:],
                                enc_projT[:, c, :].rearrange("p (b q) -> p b q", b=BL),
                                decT[:, c, :, None].broadcast_to([128, BL, P]), Add)
                            eT = eTp.tile([128, BP], F16, tag="eT",
                                          name=f"eT{s}_{c}")
                            nc.scalar.activation(eT[:], sT[:].rearrange("p b q -> p (b q)"),
                                                 Tanh)
                            eTs.append(eT)
                        # scores in two 2-bank PSUM groups, evacuated per group
                        for g in range(2):
                            ps_sc = ps_sc_pool.tile([1, 2, 512], F32, tag="sc",
                                                    name=f"pssc{s}_{g}")
                            for j in range(2):
                                n = 2 * g + j
                                for c in range(KC):
                                    nc.tensor.matmul(ps_sc[:, j, 0:BP // KC],
                                                     wf_sb[:, c, :],
                                                     eTs[c][:, bass.ts(n, BP // KC)],
                                                     start=(c == 0), stop=(c == KC - 1))
                            sc_row = scrp.tile([1, 2, BP // KC], F32, tag="scr",
                                               name=f"scrow{s}_{g}")
                            nc.vector.tensor_copy(sc_row[:], ps_sc[:, :, 0:BP // KC])
                            nc.sync.dma_start(
                                scores_buf[4 * g:4 * g + 4, 0:P],
                                sc_row[:].rearrange("o n q -> o (n q)"))

                        # 7..9) softmax, in place (|scores| < ~2, no max-shift)
                        sumexp = smp.tile([BL, 1], F32, tag="sm", name=f"sumexp{s}")
                        nc.scalar.activation(scores_buf[:, 0:P], scores_buf[:, 0:P],
                                             Exp, accum_out=sumexp[:])
                        rec = smp.tile([BL, 1], F32, tag="sm", name=f"rec{s}")
                        nc.vector.reciprocal(rec[:], sumexp[:])
                        nc.vector.tensor_scalar_mul(scores_buf[:, 0:P],
                                                    scores_buf[:, 0:P], rec[:])
                        alpha = scores_buf

                        # 10) transpose alpha (P padded to 256), one-copy
                        # scatter into block-diagonal alphaD [128, 2*BL, BL]
                        ps_tr_a = ps_small.tile([128, 2, BL], F32, tag="small",
                                                name=f"pstra{s}")
                        for j in range(2):
                            nc.tensor.transpose(ps_tr_a[:, j, :],
                                                alpha[:, 128 * j:128 * (j + 1)],
                                                ident[:])
                        src_a = ps_tr_a[:]
                        src_diag = bass.AP(
                            tensor=src_a.tensor, offset=src_a.offset,
                            ap=[list(src_a.ap[0]), [1, BL], [BL, 2]])
                        nc.vector.tensor_copy(alphaD_diag, src_diag)

                        # 11) ctx[b,:] = sum_p alpha[b,p] feats[b,p,:]
                        ps_ctx = ps_bg.tile([BL, ENC], F32, tag="bg",
                                            name=f"psctx{s}")
                        for k in range(2 * BL):
                            nc.tensor.matmul(ps_ctx[:], alphaD[:, k, :],
                                             feats16[:, k, :],
                                             start=(k == 0), stop=(k == 2 * BL - 1))

                        # 13) ctx2 = (1 + tanh(zb/2)) * ctx
                        ctx2 = t2k.tile([BL, ENC], F32, tag="t2k", name=f"ctx2{s}")
                        nc.vector.scalar_tensor_tensor(ctx2[:], taub[:], 1.0,
                                                       ps_ctx[:], op0=Add, op1=Mult)

                        # 16) ctx2T [128, KC, BL]
                        ps_tr_c = ps_small.tile([128, KC, BL], F32, tag="small",
                                                name=f"pstrc{s}")
                        for c in range(KC):
                            nc.tensor.transpose(ps_tr_c[:, c, :],
                                                ctx2[:, bass.ts(c, 128)], ident[:])
                        ctx2T = tinyp.tile([128, KC, BL], BF16, tag="tiny",
                                           name=f"ctx2T{s}")
                        nc.scalar.activation(ctx2T[:], ps_tr_c[:], Ident)

                        # 17) finish gates; n=2,3 do h-part + emb here too
                        gacts = []  # tau_i, tau_f, gt, tau_o
                        for n in range(4):
                            nsl = bass.ts(n, D)
                            if n >= 2:
                                for kc in range(KC):
                                    nc.tensor.matmul(ps_g[n][:], hT[:, kc, :],
                                                     whh_sb[:, kc, nsl],
                                                     start=(kc == 0), stop=False)
                                nc.tensor.matmul(ps_g[n][:],
                                                 idsel[:, erow:erow + BL],
                                                 emb_pre[:, s // 16, nsl],
                                                 start=False, stop=False)
                            for kc in range(KC):
                                nc.tensor.matmul(ps_g[n][:], ctx2T[:, kc, :],
                                                 wih2_sb[:, kc, nsl],
                                                 start=False, stop=(kc == KC - 1))
                            ga = t2k.tile([BL, D], F32, tag="ga", name=f"ga{s}_{n}")
                            nc.scalar.activation(ga[:], ps_g[n][:], Tanh,
                                                 scale=(1.0 if n == 2 else 0.5))
                            gacts.append(ga)
                        tau_i, tau_f, gt, tau_o = gacts

                        # 22..27) LSTM cell in tanh form
                        t_a = t2k.tile([BL, D], F32, tag="t2k", name=f"ta{s}")
                        nc.vector.scalar_tensor_tensor(t_a[:], tau_f[:], 1.0,
                                                       c_prev[:], op0=Add, op1=Mult)
                        t_b = t2k.tile([BL, D], F32, tag="t2k", name=f"tb{s}")
                        nc.vector.scalar_tensor_tensor(t_b[:], tau_i[:], 1.0,
                                                       gt[:], op0=Add, op1=Mult)
                        s2 = t2k.tile([BL, D], F32, tag="t2k", name=f"s2{s}")
                        nc.vector.tensor_add(s2[:], t_a[:], t_b[:])
                        nc.vector.tensor_scalar_mul(c_next[:], s2[:], 0.5)
                        tc2 = t2k.tile([BL, D], F32, tag="t2k", name=f"tc2{s}")
                        nc.scalar.activation(tc2[:], s2[:], Tanh, scale=0.5)
                        h2 = t2k.tile([BL, D], F32, tag="t2k", name=f"h2{s}")
                        nc.vector.scalar_tensor_tensor(h2[:], tau_o[:], 1.0, tc2[:],
                                                       op0=Add, op1=Mult)

                        # 28) h2 -> H_all / H16 slot s+1
                        ps_tr_h = ps_small.tile([128, KC, BL], F32, tag="small",
                                                name=f"pstrh{s}")
                        for c in range(KC):
                            nc.tensor.transpose(ps_tr_h[:, c, :],
                                                h2[:, bass.ts(c, 128)], ident[:])
                        nc.vector.tensor_copy(H_all[:, :, s + 1, :], ps_tr_h[:])
                        nc.scalar.activation(H16[:, :, s + 1, :], ps_tr_h[:], Ident)

                        # interleaved fc pass 1 (t=1..16) once its H is final
                        if s >= FC_S0:
                            for _ in range(FC_PER_STEP):
                                if fc_next < len(V_CHUNKS):
                                    fc_chunk(fc_next, 1, 13, fcw1, "fcw1")
                                    fc_next += 1

                # ---------------- fc pass 2 (t=17..29) ----------------
                for ci in range(len(V_CHUNKS)):
                    fc_chunk(ci, 14, 16, fcw2, "fcw2")

    nc.compile()
    return nc


def _prep_core_inputs(inputs, k):
    """Host-side marshalling for core k (samples 8k..8k+8)."""
    f32 = np.float32
    bs = slice(BL * k, BL * (k + 1))
    feats = np.ascontiguousarray(inputs["encoder_feats"][bs]).astype(f32)
    pooled = np.ascontiguousarray(inputs["encoder_pooled"][bs]).astype(f32)
    caps = np.asarray(inputs["captions"][bs])

    import ml_dtypes

    d = {}
    ft = feats.transpose(2, 0, 1).reshape(KC, 128, BP)
    d["featsT"] = np.ascontiguousarray(ft.transpose(1, 0, 2)).astype(
        ml_dtypes.bfloat16)
    fp = np.zeros((128, 2 * BL, ENC), f32)
    for b in range(BL):
        fp[0:128, 2 * b] = feats[b, 0:128]
        fp[0:P - 128, 2 * b + 1] = feats[b, 128:P]
    d["feats16"] = fp.astype(ml_dtypes.bfloat16)
    d["pooledT"] = np.ascontiguousarray(
        pooled.T.reshape(KC, 128, BL).transpose(1, 0, 2))
    emb = np.asarray(inputs["emb_table"], f32)[caps[:, :S]]      # (8, 29, 512)
    et = np.zeros((128, KC, 32, BL), f32)
    g = emb.transpose(2, 1, 0).reshape(KC, 128, S, BL)
    et[:, :, 0:S, :] = g.transpose(1, 0, 2, 3)
    d["embT"] = et.astype(ml_dtypes.bfloat16)
    return d


def _prep_shared_inputs(inputs):
    f32 = np.float32
    import ml_dtypes
    bf16 = ml_dtypes.bfloat16

    def rows(w):  # (512, N) -> [128, 4, N]
        return np.ascontiguousarray(
            np.asarray(w, f32).reshape(KC, 128, -1).transpose(1, 0, 2))

    d = {}
    d["Wd16"] = rows(0.5 * np.asarray(inputs["Wd_att"], f32)).astype(bf16)
    d["wf16"] = rows(inputs["wf_att"]).astype(np.float16)
    d["Wih2"] = rows(0.5 * np.asarray(inputs["W_ih"][E:], f32)).astype(bf16)
    d["Whh"] = rows(0.5 * np.asarray(inputs["W_hh"], f32))
    d["WihE"] = rows(inputs["W_ih"][:E]).astype(bf16)
    d["Wbeta"] = rows(0.5 * np.asarray(inputs["W_beta"], f32)).astype(bf16)
    d["Winih"] = rows(inputs["W_init_h"])
    d["Winic"] = rows(inputs["W_init_c"])
    d["We"] = rows(inputs["We_att"]).astype(bf16)
    d["Wfc16"] = rows(0.5 * np.asarray(inputs["W_fc"], f32)).astype(bf16)
    d["bihh"] = (np.asarray(inputs["b_ih"], f32)
                 + np.asarray(inputs["b_hh"], f32)).reshape(1, -1)
    d["binih"] = np.asarray(inputs["b_init_h"], f32).reshape(1, -1)
    d["binic"] = np.asarray(inputs["b_init_c"], f32).reshape(1, -1)
    d["bd_til"] = np.ascontiguousarray(
        np.asarray(inputs["bd_att"], f32).reshape(KC, 128).T)
    d["be_til"] = np.ascontiguousarray(
        np.asarray(inputs["be_att"], f32).reshape(KC, 128).T)
    d["ones128"] = np.ones((1, 128), f32)
    return d


_NC_CACHE = {}


def _get_program(with_biases=False):
    if with_biases not in _NC_CACHE:
        _NC_CACHE[with_biases] = build_program(with_biases)
    return _NC_CACHE[with_biases]


def run_on_device(inputs, trace=False, **kw):
    with_biases = bool(np.any(np.asarray(inputs["b_beta"], np.float32)))
    nc = _get_program(with_biases)
    shared = _prep_shared_inputs(inputs)
    if with_biases:
        shared["bbeta"] = np.asarray(inputs["b_beta"], np.float32).reshape(1, -1)
    in_maps = []
    for k in range(NCORES):
        m = dict(shared)
        m.update(_prep_core_inputs(inputs, k))
        in_maps.append(m)
    return run_bass_kernel_spmd(nc, in_maps, list(range(NCORES)), trace=trace, **kw)


def kernel(**inputs) -> np.ndarray:
    res = run_on_device(inputs)
    parts = [res.results[k]["out_logits"] for k in range(NCORES)]
    out = np.concatenate(parts, axis=0)
    b_fc = np.asarray(inputs["b_fc"], np.float32).reshape(1, 1, V)
    out[:, 1:, :] += b_fc
    return out


# revision 15
# speedup vs baseline: 1.1968x; 1.1968x over previous
"""Trainium2 Bass kernel for an LSTM decoder with additive attention + large
vocab projection (nn_DecoderWithAttention).

Strategy: 8-way data parallel over batch (8 samples per core), zero
collectives. Recurrent state h is kept feature-major [D, B] and scaled by 2
(h' = 2h) so every sigmoid can be computed as sigma(x) = (1 + tanh(x/2)) / 2
on the ACT engine -- keeping all scalar-engine ops inside the single
"exp_and_others" activation table set. The compensating 0.5 factors are
folded into W_d/W_beta/W_hh/W_fc/W_ih2 on the host (exact power-of-two
scale).

v2 changes vs v1 (1.90 ms):
  - fc phase split into two bf16 passes over H (t=1..16 and t=17..29); the
    first pass is emitted interleaved into steps 16..28 so its matmuls/DMAs
    hide in the recurrent loop's idle time. W_fc and the fc copy of h are
    bf16 (~0.4% rounding on logits only).
  - PSUM rebudgeted (8 banks: small 1 / scores 4 / ctx+fc 1 / beta+gates 2)
    so the beta matmul and ~half of the gates h-part matmuls prefill at step
    start, overlapping the attention phase.
  - Attention tensors (enc_projT / decT / sT / eT / wf) in fp16: halves
    SBUF + lets the scores matmul stream fp16.
  - alpha scatter into the block-diagonal tile is one strided tensor_copy.
  - dec_proj uses bf16 Wd x bf16 h' (1 cyc/row vs fp32's 4).

Per-core per-step dataflow (s = 0..28):
  dec_projT [A,B]  = (0.5 Wd)^T @ h'          (PE, bf16)
  beta/gates h-part matmuls prefill           (PE, during attention)
  eT[A,(B,P)]      = tanh(enc_projT + dec_projT bcast)   (DVE add, ACT tanh)
  scores[1,(B,P)]  = wf^T @ eT                (PE, fp16 streaming)
  alpha            = exp(scores)/sum          (ACT exp + fused accum)
  ctx[B,ENC]       = alpha @ feats            (PE, block-diag 16-K accum)
  ctx2             = (1 + tanh(zb/2)) * ctx   (= 2 sigmoid(zb) ctx)
  gates[B,4D]      = emb_pre[s] + ctx2@(W_ih2/2) + h'@(W_hh/2)
  LSTM cell in tanh form; h' = (1+tanh(o/2)) * tanh(c2)
Output row t=0 stays zero (buffer pre-zeroed + explicit zero DMA).
"""

import os
import sys

for _p in ("/opt/trn_rl_repo", os.path.expanduser("~/.axon_site/_ro/trn_rl_repo")):
    if os.path.isdir(_p) and _p not in sys.path:
        sys.path.insert(0, _p)

import numpy as np

import concourse.bass as bass
import concourse.tile as tile
from concourse import bacc, mybir
from concourse.bass_utils import run_bass_kernel_spmd
from concourse.masks import make_identity

F32 = mybir.dt.float32
F32R = mybir.dt.float32r
F16 = mybir.dt.float16
BF16 = mybir.dt.bfloat16

B, P, T = 64, 196, 30
E, D, A, ENC, V = 512, 512, 512, 512, 30000
NCORES = 8
BL = B // NCORES          # 8 samples per core
S = T - 1                 # 29 recurrent steps
BP = BL * P               # 1568
KC = 4                    # 128-row chunks per 512 feature dim
VCHUNK = 512
V_CHUNKS = [(i * VCHUNK, min(VCHUNK, V - i * VCHUNK))
            for i in range((V + VCHUNK - 1) // VCHUNK)]
# fc pass 1 (t=1..13, 104 rows) is interleaved into steps FC_S0..28.
FC_S0 = 13
FC_PER_STEP = 4

Tanh = mybir.ActivationFunctionType.Tanh
Exp = mybir.ActivationFunctionType.Exp
Ident = mybir.ActivationFunctionType.Identity
Add = mybir.AluOpType.add
Mult = mybir.AluOpType.mult


def r(ap):
    return ap.bitcast(F32R)


def build_program(with_biases=False):
    nc = bacc.Bacc(
        "TRN2",
        target_bir_lowering=False,
        debug=False,
        enable_asserts=False,
        num_devices=NCORES,
    )

    def din(name, shape, dt=F32):
        return nc.dram_tensor(name, list(shape), dt, kind="ExternalInput").ap()

    featsT_d = din("featsT", (128, KC, BP), BF16)          # [p,c,b*196+q] = feats[b,q,128c+p]
    feats16_d = din("feats16", (128, 2 * BL, ENC), BF16)   # (b,p) rows, P padded to 256
    pooledT_d = din("pooledT", (128, KC, BL), F32R)
    embT_d = din("embT", (128, KC, 32, BL), BF16)          # [p,c,t,b], t<29 used
    Wd16_d = din("Wd16", (128, KC, A), BF16)               # 0.5*Wd_att rows, bf16
    wf_d = din("wf16", (128, KC, 1), F16)
    Wih2_d = din("Wih2", (128, KC, 4 * D), BF16)           # 0.5*W_ih[512:] rows
    Whh_d = din("Whh", (128, KC, 4 * D), F32R)             # 0.5*W_hh rows
    WihE_d = din("WihE", (128, KC, 4 * D), BF16)           # W_ih[:512] rows
    Wbeta_d = din("Wbeta", (128, KC, ENC), BF16)           # 0.5*W_beta rows
    Winih_d = din("Winih", (128, KC, D), F32R)
    Winic_d = din("Winic", (128, KC, D), F32R)
    We_d = din("We", (128, KC, A), BF16)                   # We_att rows
    Wfc16_d = din("Wfc16", (128, KC, V), BF16)             # 0.5*W_fc rows, bf16
    bihh_d = din("bihh", (1, 4 * D), F32R)                 # b_ih + b_hh
    binih_d = din("binih", (1, D), F32R)
    binic_d = din("binic", (1, D), F32R)
    bd_d = din("bd_til", (128, KC))                        # bd_att as [p, c]
    be_d = din("be_til", (128, KC))
    ones_d = din("ones128", (1, 128), F32R)
    if with_biases:
        bbeta_d = din("bbeta", (1, ENC), F32R)

    out_d = nc.dram_tensor("out_logits", [BL, T, V], F32, kind="ExternalOutput").ap()
    outT = out_d.rearrange("b t v -> t b v")

    with tile.TileContext(nc) as tc:
        with tc.tile_pool(name="const", bufs=1) as const:
            wd_sb = const.tile([128, KC, A], BF16)
            wf_sb = const.tile([128, KC, 1], F16)
            wih2_sb = const.tile([128, KC, 4 * D], BF16)
            whh_sb = const.tile([128, KC, 4 * D], F32R)
            wbeta_sb = const.tile([128, KC, ENC], BF16)
            enc_projT = const.tile([128, KC, BP], F16)
            emb_pre = const.tile([128, 2, 4 * D], F32R)
            H_all = const.tile([128, KC, T, BL], F32R)   # slot t: h' after t steps
            H16 = const.tile([128, KC, T, BL], BF16)     # bf16 copy for fc + dec
            bd_sb = const.tile([128, KC], F32)
            be_sb = const.tile([128, KC], F32)
            ident = const.tile([BL, BL], F32)
            ident128 = const.tile([128, 128], F32)
            idsel = const.tile([128, 128], F32R)
            c_state = [const.tile([BL, D], F32, tag=f"cstate{i}", name=f"c_state{i}")
                       for i in range(2)]
            if with_biases:
                ones_lp = const.tile([1, 128], F32R)
                bbeta_sb = const.tile([1, ENC], F32R)
                nc.sync.dma_start(ones_lp[:], ones_d)
                nc.sync.dma_start(bbeta_sb[:], bbeta_d)

            nc.sync.dma_start(wd_sb[:], Wd16_d)
            nc.sync.dma_start(wf_sb[:], wf_d)
            nc.sync.dma_start(wih2_sb[:], Wih2_d)
            nc.sync.dma_start(whh_sb[:], Whh_d)
            nc.sync.dma_start(wbeta_sb[:], Wbeta_d)
            nc.sync.dma_start(bd_sb[:], bd_d)
            nc.sync.dma_start(be_sb[:], be_d)
            make_identity(nc, ident[:])
            make_identity(nc, ident128[:])
            nc.vector.tensor_copy(idsel[:], ident128[:])

            # ---------------- setup phase ----------------
            with tc.tile_pool(name="setup", bufs=1) as setup, \
                 tc.tile_pool(name="setup2", bufs=2) as setup2, \
                 tc.tile_pool(name="setup_ps", bufs=2, space="PSUM") as setup_ps:

                pooledT_sb = setup.tile([128, KC, BL], F32R)
                ones_sb = setup.tile([1, 128], F32R)
                bihh_sb = setup.tile([1, 4 * D], F32R)
                binih_sb = setup.tile([1, D], F32R)
                binic_sb = setup.tile([1, D], F32R)
                winih_sb = setup.tile([128, KC, D], F32R)
                winic_sb = setup.tile([128, KC, D], F32R)
                nc.sync.dma_start(pooledT_sb[:], pooledT_d)
                nc.sync.dma_start(ones_sb[:], ones_d)
                nc.sync.dma_start(bihh_sb[:], bihh_d)
                nc.sync.dma_start(binih_sb[:], binih_d)
                nc.sync.dma_start(binic_sb[:], binic_d)
                nc.sync.dma_start(winih_sb[:], Winih_d)
                nc.sync.dma_start(winic_sb[:], Winic_d)

                # h0/c0 (B-major): lhsT = pooledT chunks, rhs = W_init rows
                for which in range(2):
                    w_sb = winih_sb if which == 0 else winic_sb
                    b_row = binih_sb if which == 0 else binic_sb
                    ps = setup_ps.tile([BL, D], F32, tag="init_ps")
                    for kc in range(KC):
                        nc.tensor.matmul(ps[:], pooledT_sb[:, kc, :],
                                         w_sb[:, kc, :], start=(kc == 0), stop=False)
                    nc.tensor.matmul(ps[:], ones_sb[0:1, 0:BL], b_row[0:1, :],
                                     start=False, stop=True)
                    if which == 0:
                        h0 = setup.tile([BL, D], F32)
                        nc.scalar.activation(h0[:], ps[:], Tanh)
                        h0x2 = setup.tile([BL, D], F32)
                        nc.vector.tensor_scalar_mul(h0x2[:], h0[:], 2.0)
                        trps = setup_ps.tile([128, KC, BL], F32, tag="tr_ps")
                        for c in range(KC):
                            nc.tensor.transpose(trps[:, c, :],
                                                h0x2[:, c * 128:(c + 1) * 128],
                                                ident[:])
                        nc.vector.tensor_copy(H_all[:, :, 0, :], trps[:])
                        nc.scalar.activation(H16[:, :, 0, :], trps[:], Ident)
                    else:
                        nc.scalar.activation(c_state[0][:], ps[:], Tanh)

                # enc_projT = We^T @ featsT + be  (A-major, fp16 out)
                we_sb = setup.tile([128, KC, A], BF16)
                nc.sync.dma_start(we_sb[:], We_d)
                for n in range(KC):
                    nsl = bass.ts(n, BP // KC)  # 392 cols
                    ft_stage = setup2.tile([128, KC, BP // KC], BF16, tag="ftst")
                    nc.sync.dma_start(ft_stage[:], featsT_d[:, :, nsl])
                    for c in range(KC):
                        ps = setup_ps.tile([128, BP // KC], F32, tag="enc_ps")
                        for kc in range(KC):
                            nc.tensor.matmul(ps[:], we_sb[:, kc, bass.ts(c, 128)],
                                             ft_stage[:, kc, :],
                                             start=(kc == 0), stop=(kc == KC - 1))
                        nc.scalar.activation(enc_projT[:, c, nsl], ps[:], Ident,
                                             bias=be_sb[:, c:c + 1])

                # emb_pre = embT^T @ W_ih[:512] + (b_ih + b_hh), rows (t, b)
                nc.vector.memset(emb_pre[:].bitcast(F32), 0.0)
                embT_sb = setup.tile([128, KC, 32, BL], BF16)
                nc.sync.dma_start(embT_sb[:], embT_d)
                for n in range(4):
                    nsl = bass.ts(n, 512)
                    wst = setup2.tile([128, KC, 512], BF16, tag="wihE")
                    nc.sync.dma_start(wst[:], WihE_d[:, :, nsl])
                    for mt, (t0, nt) in enumerate([(0, 16), (16, 13)]):
                        rows = nt * BL
                        ps = setup_ps.tile([128, 512], F32, tag="emb_ps")
                        for kc in range(KC):
                            nc.tensor.matmul(ps[0:rows, :],
                                             embT_sb[:, kc, t0:t0 + nt, :],
                                             wst[:, kc, :], start=(kc == 0),
                                             stop=False)
                        nc.tensor.matmul(ps[0:rows, :], ones_sb[0:1, 0:rows],
                                         bihh_sb[0:1, nsl], start=False, stop=True)
                        nc.vector.tensor_copy(emb_pre[0:rows, mt, nsl], ps[0:rows, :])

            # ------------- recurrent loop + interleaved fc pass 1 -------------
            with tc.tile_pool(name="fcw1", bufs=3) as fcw1, \
                 tc.tile_pool(name="fcw2", bufs=6) as fcw2, \
                 tc.tile_pool(name="fco", bufs=3) as fco, \
                 tc.tile_pool(name="ps_cf", bufs=1, space="PSUM") as ps_cf:

                def fc_chunk(ci, t0, nt, wpool, wtag, pspool, pstag):
                    v0, vn = V_CHUNKS[ci]
                    rows = nt * BL
                    wst = wpool.tile([128, KC, VCHUNK], BF16, tag=wtag,
                                     name=f"fw{wtag}{ci}")
                    nc.sync.dma_start(wst[:, :, 0:vn], Wfc16_d[:, :, v0:v0 + vn])
                    ps = pspool.tile([128, VCHUNK], F32, tag=pstag,
                                     name=f"psfc{t0}_{ci}")
                    for kc in range(KC):
                        nc.tensor.matmul(ps[0:rows, 0:vn],
                                         H16[:, kc, t0:t0 + nt, :],
                                         wst[:, kc, 0:vn],
                                         start=(kc == 0), stop=(kc == KC - 1))
                    ost = fco.tile([128, VCHUNK], F32, tag="fco",
                                   name=f"fo{t0}_{ci}")
                    nc.vector.tensor_copy(ost[0:rows, 0:vn], ps[0:rows, 0:vn])
                    nc.sync.dma_start(outT[t0:t0 + nt, :, v0:v0 + vn],
                                      ost[0:rows, 0:vn])

                # output row t=0 is defined to be zeros
                zt = fco.tile([1, 400], F32, tag="zt")
                nc.vector.memset(zt[:], 0.0)
                zsrc = zt[0:1, 0:400]
                zsrc = bass.AP(tensor=zsrc.tensor, offset=zsrc.offset,
                               ap=[list(zsrc.ap[0]), [0, BL * V // 400], [1, 400]])
                nc.sync.dma_start(out_d[:, 0, :], zsrc)

                with tc.tile_pool(name="lper", bufs=1) as lper, \
                     tc.tile_pool(name="sT", bufs=2) as sTp, \
                     tc.tile_pool(name="eT", bufs=2) as eTp, \
                     tc.tile_pool(name="scr", bufs=2) as scrp, \
                     tc.tile_pool(name="t2k", bufs=4) as t2k, \
                     tc.tile_pool(name="tiny", bufs=3) as tinyp, \
                     tc.tile_pool(name="sm", bufs=2) as smp, \
                     tc.tile_pool(name="ps_small", bufs=1, space="PSUM") as ps_small, \
                     tc.tile_pool(name="ps_sc", bufs=1, space="PSUM") as ps_sc_pool, \
                     tc.tile_pool(name="ps_bg", bufs=2, space="PSUM") as ps_bg:

                    feats16 = lper.tile([128, 2 * BL, ENC], BF16)
                    nc.sync.dma_start(feats16[:], feats16_d)
                    alphaD = lper.tile([128, 2 * BL, BL], BF16)
                    nc.vector.memset(alphaD[:], 0.0)
                    scores_buf = lper.tile([BL, 256], F32)
                    nc.vector.memset(scores_buf[:, P:256], 0.0)
                    # strided view of alphaD for the one-copy block-diag
                    # scatter: dst[b, j] = alphaD[:, 2b+j, b]
                    aD0 = alphaD[:]
                    alphaD_diag = bass.AP(
                        tensor=aD0.tensor, offset=aD0.offset,
                        ap=[list(aD0.ap[0]), [2 * BL + 1, BL], [BL, 2]])

                    fc_next = 0
                    for s in range(S):
                        hT = H_all[:, :, s, :]
                        c_prev = c_state[s % 2]
                        c_next = c_state[(s + 1) % 2]

                        # 1) dec_projT [128, KC, BL] (bf16 Wd x bf16 h')
                        ps_dec = ps_small.tile([128, KC, BL], F32, tag="small",
                                               name=f"psdec{s}")
                        for m in range(KC):
                            for kc in range(KC):
                                nc.tensor.matmul(ps_dec[:, m, :],
                                                 wd_sb[:, kc, bass.ts(m, 128)],
                                                 H16[:, kc, s, :],
                                                 start=(kc == 0),
                                                 stop=(kc == KC - 1))
                        decT = tinyp.tile([128, KC, BL], F16, tag="tiny",
                                          name=f"decT{s}")
                        for c in range(KC):
                            nc.scalar.activation(decT[:, c, :], ps_dec[:, c, :],
                                                 Ident, bias=bd_sb[:, c:c + 1])

                        # beta gate preact (h-only): prefills during attention
                        ps_b = ps_bg.tile([BL, ENC], F32, tag="bg",
                                          name=f"psb{s}")
                        for kc in range(KC):
                            nc.tensor.matmul(ps_b[:], H16[:, kc, s, :],
                                             wbeta_sb[:, kc, :],
                                             start=(kc == 0),
                                             stop=(not with_biases and kc == KC - 1))
                        if with_biases:
                            nc.tensor.matmul(ps_b[:], ones_lp[0:1, 0:BL],
                                             bbeta_sb[0:1, :], start=False, stop=True)
                        taub = t2k.tile([BL, ENC], F32, tag="t2k", name=f"taub{s}")
                        nc.scalar.activation(taub[:], ps_b[:], Tanh, scale=0.5)

                        # gates: h-part + emb selector prefill for n=0,1
                        erow = (s % 16) * BL
                        ps_g = []
                        for n in range(4):
                            ps_g.append(ps_bg.tile([BL, D], F32, tag="bg",
                                                   name=f"psg{s}_{n}"))
                        for n in range(2):
                            nsl = bass.ts(n, D)
                            for kc in range(KC):
                                nc.tensor.matmul(ps_g[n][:], hT[:, kc, :],
                                                 whh_sb[:, kc, nsl],
                                                 start=(kc == 0), stop=False)
                            nc.tensor.matmul(ps_g[n][:],
                                             idsel[:, erow:erow + BL],
                                             emb_pre[:, s // 16, nsl],
                                             start=False, stop=False)

                        # 2..5) e = tanh(enc_proj + dec_proj); scores = wf^T e
                        # last add chunk runs on GpSimd so DVE only does 3
                        ps_sc = ps_sc_pool.tile([1, KC, 512], F32, tag="sc",
                                                name=f"pssc{s}")
                        for c in range(KC):
                            sT = sTp.tile([128, BL, P], F16, tag="sT",
                                          name=f"sT{s}_{c}")
                            eng = nc.gpsimd if c == KC - 1 else nc.vector
                            eng.tensor_tensor(
                                sT[:],
                                enc_projT[:, c, :].rearrange("p (b q) -> p b q", b=BL),
                                decT[:, c, :, None].broadcast_to([128, BL, P]), Add)
                            eT = eTp.tile([128, BP], F16, tag="eT",
                                          name=f"eT{s}_{c}")
                            nc.scalar.activation(eT[:], sT[:].rearrange("p b q -> p (b q)"),
                                                 Tanh)
                            for n in range(KC):
                                nc.tensor.matmul(ps_sc[:, n, 0:BP // KC],
                                                 wf_sb[:, c, :],
                                                 eT[:, bass.ts(n, BP // KC)],
                                                 start=(c == 0), stop=(c == KC - 1))

                        # 6) PSUM -> SBUF row, DMA-reshape to [BL, P]
                        sc_row = scrp.tile([1, KC, BP // KC], F32, tag="scr",
                                           name=f"scrow{s}")
                        nc.vector.tensor_copy(sc_row[:], ps_sc[:, :, 0:BP // KC])
                        nc.sync.dma_start(scores_buf[:, 0:P],
                                          sc_row[:].rearrange("o n q -> o (n q)"))

                        # 7..9) softmax, in place (|scores| < ~2, no max-shift)
                        sumexp = smp.tile([BL, 1], F32, tag="sm", name=f"sumexp{s}")
                        nc.scalar.activation(scores_buf[:, 0:P], scores_buf[:, 0:P],
                                             Exp, accum_out=sumexp[:])
                        rec = smp.tile([BL, 1], F32, tag="sm", name=f"rec{s}")
                        nc.vector.reciprocal(rec[:], sumexp[:])
                        nc.vector.tensor_scalar_mul(scores_buf[:, 0:P],
                                                    scores_buf[:, 0:P], rec[:])
                        alpha = scores_buf

                        # 10) transpose alpha (P padded to 256), one-copy
                        # scatter into block-diagonal alphaD [128, 2*BL, BL]
                        ps_tr_a = ps_small.tile([128, 2, BL], F32, tag="small",
                                                name=f"pstra{s}")
                        for j in range(2):
                            nc.tensor.transpose(ps_tr_a[:, j, :],
                                                alpha[:, 128 * j:128 * (j + 1)],
                                                ident[:])
                        src_a = ps_tr_a[:]
                        src_diag = bass.AP(
                            tensor=src_a.tensor, offset=src_a.offset,
                            ap=[list(src_a.ap[0]), [1, BL], [BL, 2]])
                        nc.vector.tensor_copy(alphaD_diag, src_diag)

                        # 11) ctx[b,:] = sum_p alpha[b,p] feats[b,p,:]
                        ps_ctx = ps_cf.tile([BL, ENC], F32, tag="cf",
                                            name=f"psctx{s}")
                        for k in range(2 * BL):
                            nc.tensor.matmul(ps_ctx[:], alphaD[:, k, :],
                                             feats16[:, k, :],
                                             start=(k == 0), stop=(k == 2 * BL - 1))

                        # 13) ctx2 = (1 + tanh(zb/2)) * ctx
                        ctx2 = t2k.tile([BL, ENC], F32, tag="t2k", name=f"ctx2{s}")
                        nc.vector.scalar_tensor_tensor(ctx2[:], taub[:], 1.0,
                                                       ps_ctx[:], op0=Add, op1=Mult)

                        # 16) ctx2T [128, KC, BL]
                        ps_tr_c = ps_small.tile([128, KC, BL], F32, tag="small",
                                                name=f"pstrc{s}")
                        for c in range(KC):
                            nc.tensor.transpose(ps_tr_c[:, c, :],
                                                ctx2[:, bass.ts(c, 128)], ident[:])
                        ctx2T = tinyp.tile([128, KC, BL], BF16, tag="tiny",
                                           name=f"ctx2T{s}")
                        nc.scalar.activation(ctx2T[:], ps_tr_c[:], Ident)

                        # 17) finish gates; n=2,3 do h-part + emb here too
                        gacts = []  # tau_i, tau_f, gt, tau_o
                        for n in range(4):
                            nsl = bass.ts(n, D)
                            if n >= 2:
                                for kc in range(KC):
                                    nc.tensor.matmul(ps_g[n][:], hT[:, kc, :],
                                                     whh_sb[:, kc, nsl],
                                                     start=(kc == 0), stop=False)
                                nc.tensor.matmul(ps_g[n][:],
                                                 idsel[:, erow:erow + BL],
                                                 emb_pre[:, s // 16, nsl],
                                                 start=False, stop=False)
                            for kc in range(KC):
                                nc.tensor.matmul(ps_g[n][:], ctx2T[:, kc, :],
                                                 wih2_sb[:, kc, nsl],
                                                 start=False, stop=(kc == KC - 1))
                            ga = t2k.tile([BL, D], F32, tag="ga", name=f"ga{s}_{n}")
                            nc.scalar.activation(ga[:], ps_g[n][:], Tanh,
                                                 scale=(1.0 if n == 2 else 0.5))
                            gacts.append(ga)
                        tau_i, tau_f, gt, tau_o = gacts

                        # 22..27) LSTM cell in tanh form
                        t_a = t2k.tile([BL, D], F32, tag="t2k", name=f"ta{s}")
                        nc.vector.scalar_tensor_tensor(t_a[:], tau_f[:], 1.0,
                                                       c_prev[:], op0=Add, op1=Mult)
                        t_b = t2k.tile([BL, D], F32, tag="t2k", name=f"tb{s}")
                        nc.vector.scalar_tensor_tensor(t_b[:], tau_i[:], 1.0,
                                                       gt[:], op0=Add, op1=Mult)
                        s2 = t2k.tile([BL, D], F32, tag="t2k", name=f"s2{s}")
                        nc.vector.tensor_add(s2[:], t_a[:], t_b[:])
                        nc.vector.tensor_scalar_mul(c_next[:], s2[:], 0.5)
                        tc2 = t2k.tile([BL, D], F32, tag="t2k", name=f"tc2{s}")
                        nc.scalar.activation(tc2[:], s2[:], Tanh, scale=0.5)
                        h2 = t2k.tile([BL, D], F32, tag="t2k", name=f"h2{s}")
                        nc.vector.scalar_tensor_tensor(h2[:], tau_o[:], 1.0, tc2[:],
                                                       op0=Add, op1=Mult)

                        # 28) h2 -> H_all / H16 slot s+1
                        ps_tr_h = ps_small.tile([128, KC, BL], F32, tag="small",
                                                name=f"pstrh{s}")
                        for c in range(KC):
                            nc.tensor.transpose(ps_tr_h[:, c, :],
                                                h2[:, bass.ts(c, 128)], ident[:])
                        nc.vector.tensor_copy(H_all[:, :, s + 1, :], ps_tr_h[:])
                        nc.scalar.activation(H16[:, :, s + 1, :], ps_tr_h[:], Ident)

                        # interleaved fc pass 1 (t=1..16) once its H is final
                        if s >= FC_S0:
                            for _ in range(FC_PER_STEP):
                                if fc_next < len(V_CHUNKS):
                                    fc_chunk(fc_next, 1, 13, fcw1, "fcw1",
                                             ps_sc_pool, "sc")
                                    fc_next += 1

                # ---------------- fc pass 2 (t=14..29) ----------------
                with tc.tile_pool(name="ps_fc2", bufs=3,
                                  space="PSUM") as ps_fc2:
                    for ci in range(len(V_CHUNKS)):
                        fc_chunk(ci, 14, 16, fcw2, "fcw2", ps_fc2, "fc2")

    nc.compile()
    return nc


def _prep_core_inputs(inputs, k):
    """Host-side marshalling for core k (samples 8k..8k+8)."""
    f32 = np.float32
    bs = slice(BL * k, BL * (k + 1))
    feats = np.ascontiguousarray(inputs["encoder_feats"][bs]).astype(f32)
    pooled = np.ascontiguousarray(inputs["encoder_pooled"][bs]).astype(f32)
    caps = np.asarray(inputs["captions"][bs])

    import ml_dtypes

    d = {}
    ft = feats.transpose(2, 0, 1).reshape(KC, 128, BP)
    d["featsT"] = np.ascontiguousarray(ft.transpose(1, 0, 2)).astype(
        ml_dtypes.bfloat16)
    fp = np.zeros((128, 2 * BL, ENC), f32)
    for b in range(BL):
        fp[0:128, 2 * b] = feats[b, 0:128]
        fp[0:P - 128, 2 * b + 1] = feats[b, 128:P]
    d["feats16"] = fp.astype(ml_dtypes.bfloat16)
    d["pooledT"] = np.ascontiguousarray(
        pooled.T.reshape(KC, 128, BL).transpose(1, 0, 2))
    emb = np.asarray(inputs["emb_table"], f32)[caps[:, :S]]      # (8, 29, 512)
    et = np.zeros((128, KC, 32, BL), f32)
    g = emb.transpose(2, 1, 0).reshape(KC, 128, S, BL)
    et[:, :, 0:S, :] = g.transpose(1, 0, 2, 3)
    d["embT"] = et.astype(ml_dtypes.bfloat16)
    return d


def _prep_shared_inputs(inputs):
    f32 = np.float32
    import ml_dtypes
    bf16 = ml_dtypes.bfloat16

    def rows(w):  # (512, N) -> [128, 4, N]
        return np.ascontiguousarray(
            np.asarray(w, f32).reshape(KC, 128, -1).transpose(1, 0, 2))

    d = {}
    d["Wd16"] = rows(0.5 * np.asarray(inputs["Wd_att"], f32)).astype(bf16)
    d["wf16"] = rows(inputs["wf_att"]).astype(np.float16)
    d["Wih2"] = rows(0.5 * np.asarray(inputs["W_ih"][E:], f32)).astype(bf16)
    d["Whh"] = rows(0.5 * np.asarray(inputs["W_hh"], f32))
    d["WihE"] = rows(inputs["W_ih"][:E]).astype(bf16)
    d["Wbeta"] = rows(0.5 * np.asarray(inputs["W_beta"], f32)).astype(bf16)
    d["Winih"] = rows(inputs["W_init_h"])
    d["Winic"] = rows(inputs["W_init_c"])
    d["We"] = rows(inputs["We_att"]).astype(bf16)
    d["Wfc16"] = rows(0.5 * np.asarray(inputs["W_fc"], f32)).astype(bf16)
    d["bihh"] = (np.asarray(inputs["b_ih"], f32)
                 + np.asarray(inputs["b_hh"], f32)).reshape(1, -1)
    d["binih"] = np.asarray(inputs["b_init_h"], f32).reshape(1, -1)
    d["binic"] = np.asarray(inputs["b_init_c"], f32).reshape(1, -1)
    d["bd_til"] = np.ascontiguousarray(
        np.asarray(inputs["bd_att"], f32).reshape(KC, 128).T)
    d["be_til"] = np.ascontiguousarray(
        np.asarray(inputs["be_att"], f32).reshape(KC, 128).T)
    d["ones128"] = np.ones((1, 128), f32)
    return d


_NC_CACHE = {}


def _get_program(with_biases=False):
    if with_biases not in _NC_CACHE:
        _NC_CACHE[with_biases] = build_program(with_biases)
    return _NC_CACHE[with_biases]


def run_on_device(inputs, trace=False, **kw):
    with_biases = bool(np.any(np.asarray(inputs["b_beta"], np.float32)))
    nc = _get_program(with_biases)
    shared = _prep_shared_inputs(inputs)
    if with_biases:
        shared["bbeta"] = np.asarray(inputs["b_beta"], np.float32).reshape(1, -1)
    in_maps = []
    for k in range(NCORES):
        m = dict(shared)
        m.update(_prep_core_inputs(inputs, k))
        in_maps.append(m)
    return run_bass_kernel_spmd(nc, in_maps, list(range(NCORES)), trace=trace, **kw)


def kernel(**inputs) -> np.ndarray:
    res = run_on_device(inputs)
    parts = [res.results[k]["out_logits"] for k in range(NCORES)]
    out = np.concatenate(parts, axis=0)
    b_fc = np.asarray(inputs["b_fc"], np.float32).reshape(1, 1, V)
    out[:, 1:, :] += b_fc
    return out


# revision 17
# speedup vs baseline: 1.2019x; 1.0042x over previous
"""Trainium2 Bass kernel for an LSTM decoder with additive attention + large
vocab projection (nn_DecoderWithAttention).

Strategy: 8-way data parallel over batch (8 samples per core), zero
collectives. Recurrent state h is kept feature-major [D, B] and scaled by 2
(h' = 2h) so every sigmoid can be computed as sigma(x) = (1 + tanh(x/2)) / 2
on the ACT engine -- keeping all scalar-engine ops inside the single
"exp_and_others" activation table set. The compensating 0.5 factors are
folded into W_d/W_beta/W_hh/W_fc/W_ih2 on the host (exact power-of-two
scale).

v2 changes vs v1 (1.90 ms):
  - fc phase split into two bf16 passes over H (t=1..16 and t=17..29); the
    first pass is emitted interleaved into steps 16..28 so its matmuls/DMAs
    hide in the recurrent loop's idle time. W_fc and the fc copy of h are
    bf16 (~0.4% rounding on logits only).
  - PSUM rebudgeted (8 banks: small 1 / scores 4 / ctx+fc 1 / beta+gates 2)
    so the beta matmul and ~half of the gates h-part matmuls prefill at step
    start, overlapping the attention phase.
  - Attention tensors (enc_projT / decT / sT / eT / wf) in fp16: halves
    SBUF + lets the scores matmul stream fp16.
  - alpha scatter into the block-diagonal tile is one strided tensor_copy.
  - dec_proj uses bf16 Wd x bf16 h' (1 cyc/row vs fp32's 4).

Per-core per-step dataflow (s = 0..28):
  dec_projT [A,B]  = (0.5 Wd)^T @ h'          (PE, bf16)
  beta/gates h-part matmuls prefill           (PE, during attention)
  eT[A,(B,P)]      = tanh(enc_projT + dec_projT bcast)   (DVE add, ACT tanh)
  scores[1,(B,P)]  = wf^T @ eT                (PE, fp16 streaming)
  alpha            = exp(scores)/sum          (ACT exp + fused accum)
  ctx[B,ENC]       = alpha @ feats            (PE, block-diag 16-K accum)
  ctx2             = (1 + tanh(zb/2)) * ctx   (= 2 sigmoid(zb) ctx)
  gates[B,4D]      = emb_pre[s] + ctx2@(W_ih2/2) + h'@(W_hh/2)
  LSTM cell in tanh form; h' = (1+tanh(o/2)) * tanh(c2)
Output row t=0 stays zero (buffer pre-zeroed + explicit zero DMA).
"""

import os
import sys

for _p in ("/opt/trn_rl_repo", os.path.expanduser("~/.axon_site/_ro/trn_rl_repo")):
    if os.path.isdir(_p) and _p not in sys.path:
        sys.path.insert(0, _p)

import numpy as np

import concourse.bass as bass
import concourse.tile as tile
from concourse import bacc, mybir
from concourse.bass_utils import run_bass_kernel_spmd
from concourse.masks import make_identity

F32 = mybir.dt.float32
F32R = mybir.dt.float32r
F16 = mybir.dt.float16
BF16 = mybir.dt.bfloat16

B, P, T = 64, 196, 30
E, D, A, ENC, V = 512, 512, 512, 512, 30000
NCORES = 8
BL = B // NCORES          # 8 samples per core
S = T - 1                 # 29 recurrent steps
BP = BL * P               # 1568
KC = 4                    # 128-row chunks per 512 feature dim
VCHUNK = 512
V_CHUNKS = [(i * VCHUNK, min(VCHUNK, V - i * VCHUNK))
            for i in range((V + VCHUNK - 1) // VCHUNK)]
# fc pass 1 (t=1..13, 104 rows) is interleaved into steps FC_S0..28.
FC_S0 = 13
FC_PER_STEP = 4

Tanh = mybir.ActivationFunctionType.Tanh
Exp = mybir.ActivationFunctionType.Exp
Ident = mybir.ActivationFunctionType.Identity
Add = mybir.AluOpType.add
Mult = mybir.AluOpType.mult


def r(ap):
    return ap.bitcast(F32R)


def build_program(with_biases=False):
    nc = bacc.Bacc(
        "TRN2",
        target_bir_lowering=False,
        debug=False,
        enable_asserts=False,
        num_devices=NCORES,
    )

    def din(name, shape, dt=F32):
        return nc.dram_tensor(name, list(shape), dt, kind="ExternalInput").ap()

    featsT_d = din("featsT", (128, KC, BP), BF16)          # [p,c,b*196+q] = feats[b,q,128c+p]
    feats16_d = din("feats16", (128, 2 * BL, ENC), BF16)   # (b,p) rows, P padded to 256
    pooledT_d = din("pooledT", (128, KC, BL), F32R)
    embT_d = din("embT", (128, KC, 32, BL), BF16)          # [p,c,t,b], t<29 used
    Wd16_d = din("Wd16", (128, KC, A), BF16)               # 0.5*Wd_att rows, bf16
    wf_d = din("wf16", (128, KC, 1), F16)
    Wih2_d = din("Wih2", (128, KC, 4 * D), BF16)           # 0.5*W_ih[512:] rows
    Whh_d = din("Whh", (128, KC, 4 * D), F32R)             # 0.5*W_hh rows
    WihE_d = din("WihE", (128, KC, 4 * D), BF16)           # W_ih[:512] rows
    Wbeta_d = din("Wbeta", (128, KC, ENC), BF16)           # 0.5*W_beta rows
    Winih_d = din("Winih", (128, KC, D), F32R)
    Winic_d = din("Winic", (128, KC, D), F32R)
    We_d = din("We", (128, KC, A), BF16)                   # We_att rows
    Wfc16_d = din("Wfc16", (128, KC, V), BF16)             # 0.5*W_fc rows, bf16
    bihh_d = din("bihh", (1, 4 * D), F32R)                 # b_ih + b_hh
    binih_d = din("binih", (1, D), F32R)
    binic_d = din("binic", (1, D), F32R)
    bd_d = din("bd_til", (128, KC))                        # bd_att as [p, c]
    be_d = din("be_til", (128, KC))
    ones_d = din("ones128", (1, 128), F32R)
    if with_biases:
        bbeta_d = din("bbeta", (1, ENC), F32R)

    out_d = nc.dram_tensor("out_logits", [BL, T, V], F32, kind="ExternalOutput").ap()
    outT = out_d.rearrange("b t v -> t b v")

    with tile.TileContext(nc) as tc:
        with tc.tile_pool(name="const", bufs=1) as const:
            wd_sb = const.tile([128, KC, A], BF16)
            wf_sb = const.tile([128, KC, 1], F16)
            wih2_sb = const.tile([128, KC, 4 * D], BF16)
            whh_sb = const.tile([128, KC, 4 * D], F32R)
            wbeta_sb = const.tile([128, KC, ENC], BF16)
            enc_projT = const.tile([128, KC, BP], F16)
            emb_pre = const.tile([128, 2, 4 * D], F32R)
            H_all = const.tile([128, KC, T, BL], F32R)   # slot t: h' after t steps
            H16 = const.tile([128, KC, T, BL], BF16)     # bf16 copy for fc + dec
            bd_sb = const.tile([128, KC], F32)
            be_sb = const.tile([128, KC], F32)
            ident = const.tile([BL, BL], F32)
            ident128 = const.tile([128, 128], F32)
            idsel = const.tile([128, 128], F32R)
            c_state = [const.tile([BL, D], F32, tag=f"cstate{i}", name=f"c_state{i}")
                       for i in range(2)]
            if with_biases:
                ones_lp = const.tile([1, 128], F32R)
                bbeta_sb = const.tile([1, ENC], F32R)
                nc.sync.dma_start(ones_lp[:], ones_d)
                nc.sync.dma_start(bbeta_sb[:], bbeta_d)

            nc.sync.dma_start(wd_sb[:], Wd16_d)
            nc.sync.dma_start(wf_sb[:], wf_d)
            nc.sync.dma_start(wih2_sb[:], Wih2_d)
            nc.sync.dma_start(whh_sb[:], Whh_d)
            nc.sync.dma_start(wbeta_sb[:], Wbeta_d)
            nc.sync.dma_start(bd_sb[:], bd_d)
            nc.sync.dma_start(be_sb[:], be_d)
            make_identity(nc, ident[:])
            make_identity(nc, ident128[:])
            nc.vector.tensor_copy(idsel[:], ident128[:])

            # ---------------- setup phase ----------------
            with tc.tile_pool(name="setup", bufs=1) as setup, \
                 tc.tile_pool(name="setup2", bufs=2) as setup2, \
                 tc.tile_pool(name="setup_ps", bufs=2, space="PSUM") as setup_ps:

                pooledT_sb = setup.tile([128, KC, BL], F32R)
                ones_sb = setup.tile([1, 128], F32R)
                bihh_sb = setup.tile([1, 4 * D], F32R)
                binih_sb = setup.tile([1, D], F32R)
                binic_sb = setup.tile([1, D], F32R)
                winih_sb = setup.tile([128, KC, D], F32R)
                winic_sb = setup.tile([128, KC, D], F32R)
                nc.sync.dma_start(pooledT_sb[:], pooledT_d)
                nc.sync.dma_start(ones_sb[:], ones_d)
                nc.sync.dma_start(bihh_sb[:], bihh_d)
                nc.sync.dma_start(binih_sb[:], binih_d)
                nc.sync.dma_start(binic_sb[:], binic_d)
                nc.sync.dma_start(winih_sb[:], Winih_d)
                nc.sync.dma_start(winic_sb[:], Winic_d)

                # h0/c0 (B-major): lhsT = pooledT chunks, rhs = W_init rows
                for which in range(2):
                    w_sb = winih_sb if which == 0 else winic_sb
                    b_row = binih_sb if which == 0 else binic_sb
                    ps = setup_ps.tile([BL, D], F32, tag="init_ps")
                    for kc in range(KC):
                        nc.tensor.matmul(ps[:], pooledT_sb[:, kc, :],
                                         w_sb[:, kc, :], start=(kc == 0), stop=False)
                    nc.tensor.matmul(ps[:], ones_sb[0:1, 0:BL], b_row[0:1, :],
                                     start=False, stop=True)
                    if which == 0:
                        h0 = setup.tile([BL, D], F32)
                        nc.scalar.activation(h0[:], ps[:], Tanh)
                        h0x2 = setup.tile([BL, D], F32)
                        nc.vector.tensor_scalar_mul(h0x2[:], h0[:], 2.0)
                        trps = setup_ps.tile([128, KC, BL], F32, tag="tr_ps")
                        for c in range(KC):
                            nc.tensor.transpose(trps[:, c, :],
                                                h0x2[:, c * 128:(c + 1) * 128],
                                                ident[:])
                        nc.vector.tensor_copy(H_all[:, :, 0, :], trps[:])
                        nc.scalar.activation(H16[:, :, 0, :], trps[:], Ident)
                    else:
                        nc.scalar.activation(c_state[0][:], ps[:], Tanh)

                # enc_projT = We^T @ featsT + be  (A-major, fp16 out)
                we_sb = setup.tile([128, KC, A], BF16)
                nc.sync.dma_start(we_sb[:], We_d)
                for n in range(KC):
                    nsl = bass.ts(n, BP // KC)  # 392 cols
                    ft_stage = setup2.tile([128, KC, BP // KC], BF16, tag="ftst")
                    nc.sync.dma_start(ft_stage[:], featsT_d[:, :, nsl])
                    for c in range(KC):
                        ps = setup_ps.tile([128, BP // KC], F32, tag="enc_ps")
                        for kc in range(KC):
                            nc.tensor.matmul(ps[:], we_sb[:, kc, bass.ts(c, 128)],
                                             ft_stage[:, kc, :],
                                             start=(kc == 0), stop=(kc == KC - 1))
                        nc.scalar.activation(enc_projT[:, c, nsl], ps[:], Ident,
                                             bias=be_sb[:, c:c + 1])

                # emb_pre = embT^T @ W_ih[:512] + (b_ih + b_hh), rows (t, b)
                nc.vector.memset(emb_pre[:].bitcast(F32), 0.0)
                embT_sb = setup.tile([128, KC, 32, BL], BF16)
                nc.sync.dma_start(embT_sb[:], embT_d)
                for n in range(4):
                    nsl = bass.ts(n, 512)
                    wst = setup2.tile([128, KC, 512], BF16, tag="wihE")
                    nc.sync.dma_start(wst[:], WihE_d[:, :, nsl])
                    for mt, (t0, nt) in enumerate([(0, 16), (16, 13)]):
                        rows = nt * BL
                        ps = setup_ps.tile([128, 512], F32, tag="emb_ps")
                        for kc in range(KC):
                            nc.tensor.matmul(ps[0:rows, :],
                                             embT_sb[:, kc, t0:t0 + nt, :],
                                             wst[:, kc, :], start=(kc == 0),
                                             stop=False)
                        nc.tensor.matmul(ps[0:rows, :], ones_sb[0:1, 0:rows],
                                         bihh_sb[0:1, nsl], start=False, stop=True)
                        nc.vector.tensor_copy(emb_pre[0:rows, mt, nsl], ps[0:rows, :])

            # ------------- recurrent loop + interleaved fc pass 1 -------------
            with tc.tile_pool(name="fcw1", bufs=3) as fcw1, \
                 tc.tile_pool(name="fcw2", bufs=7) as fcw2, \
                 tc.tile_pool(name="fco", bufs=3) as fco, \
                 tc.tile_pool(name="ps_cf", bufs=1, space="PSUM") as ps_cf:

                def fc_chunk(ci, t0, nt, wpool, wtag, pspool, pstag):
                    v0, vn = V_CHUNKS[ci]
                    rows = nt * BL
                    wst = wpool.tile([128, KC, VCHUNK], BF16, tag=wtag,
                                     name=f"fw{wtag}{ci}")
                    nc.sync.dma_start(wst[:, :, 0:vn], Wfc16_d[:, :, v0:v0 + vn])
                    ps = pspool.tile([128, VCHUNK], F32, tag=pstag,
                                     name=f"psfc{t0}_{ci}")
                    for kc in range(KC):
                        nc.tensor.matmul(ps[0:rows, 0:vn],
                                         H16[:, kc, t0:t0 + nt, :],
                                         wst[:, kc, 0:vn],
                                         start=(kc == 0), stop=(kc == KC - 1))
                    ost = fco.tile([128, VCHUNK], F32, tag="fco",
                                   name=f"fo{t0}_{ci}")
                    nc.vector.tensor_copy(ost[0:rows, 0:vn], ps[0:rows, 0:vn])
                    nc.sync.dma_start(outT[t0:t0 + nt, :, v0:v0 + vn],
                                      ost[0:rows, 0:vn])

                # output row t=0 is defined to be zeros
                zt = fco.tile([1, 400], F32, tag="zt")
                nc.vector.memset(zt[:], 0.0)
                zsrc = zt[0:1, 0:400]
                zsrc = bass.AP(tensor=zsrc.tensor, offset=zsrc.offset,
                               ap=[list(zsrc.ap[0]), [0, BL * V // 400], [1, 400]])
                nc.sync.dma_start(out_d[:, 0, :], zsrc)

                with tc.tile_pool(name="lper", bufs=1) as lper, \
                     tc.tile_pool(name="sT", bufs=2) as sTp, \
                     tc.tile_pool(name="eT", bufs=2) as eTp, \
                     tc.tile_pool(name="scr", bufs=2) as scrp, \
                     tc.tile_pool(name="t2k", bufs=4) as t2k, \
                     tc.tile_pool(name="tiny", bufs=3) as tinyp, \
                     tc.tile_pool(name="sm", bufs=2) as smp, \
                     tc.tile_pool(name="ps_small", bufs=1, space="PSUM") as ps_small, \
                     tc.tile_pool(name="ps_sc", bufs=1, space="PSUM") as ps_sc_pool, \
                     tc.tile_pool(name="ps_bg", bufs=2, space="PSUM") as ps_bg:

                    feats16 = lper.tile([128, 2 * BL, ENC], BF16)
                    nc.sync.dma_start(feats16[:], feats16_d)
                    alphaD = lper.tile([128, 2 * BL, BL], BF16)
                    nc.vector.memset(alphaD[:], 0.0)
                    scores_buf = lper.tile([BL, 256], F32)
                    nc.vector.memset(scores_buf[:, P:256], 0.0)
                    # strided view of alphaD for the one-copy block-diag
                    # scatter: dst[b, j] = alphaD[:, 2b+j, b]
                    aD0 = alphaD[:]
                    alphaD_diag = bass.AP(
                        tensor=aD0.tensor, offset=aD0.offset,
                        ap=[list(aD0.ap[0]), [2 * BL + 1, BL], [BL, 2]])

                    fc_next = 0
                    for s in range(S):
                        hT = H_all[:, :, s, :]
                        c_prev = c_state[s % 2]
                        c_next = c_state[(s + 1) % 2]

                        # 1) dec_projT [128, KC, BL] (bf16 Wd x bf16 h')
                        ps_dec = ps_small.tile([128, KC, BL], F32, tag="small",
                                               name=f"psdec{s}")
                        for m in range(KC):
                            for kc in range(KC):
                                nc.tensor.matmul(ps_dec[:, m, :],
                                                 wd_sb[:, kc, bass.ts(m, 128)],
                                                 H16[:, kc, s, :],
                                                 start=(kc == 0),
                                                 stop=(kc == KC - 1))
                        decT = tinyp.tile([128, KC, BL], F16, tag="tiny",
                                          name=f"decT{s}")
                        for c in range(KC):
                            nc.scalar.activation(decT[:, c, :], ps_dec[:, c, :],
                                                 Ident, bias=bd_sb[:, c:c + 1])

                        # beta gate preact (h-only): prefills during attention
                        ps_b = ps_bg.tile([BL, ENC], F32, tag="bg",
                                          name=f"psb{s}")
                        for kc in range(KC):
                            nc.tensor.matmul(ps_b[:], H16[:, kc, s, :],
                                             wbeta_sb[:, kc, :],
                                             start=(kc == 0),
                                             stop=(not with_biases and kc == KC - 1))
                        if with_biases:
                            nc.tensor.matmul(ps_b[:], ones_lp[0:1, 0:BL],
                                             bbeta_sb[0:1, :], start=False, stop=True)
                        taub = t2k.tile([BL, ENC], F32, tag="t2k", name=f"taub{s}")
                        nc.scalar.activation(taub[:], ps_b[:], Tanh, scale=0.5)

                        # gates: h-part + emb selector prefill for n=0,1
                        erow = (s % 16) * BL
                        ps_g = []
                        for n in range(4):
                            ps_g.append(ps_bg.tile([BL, D], F32, tag="bg",
                                                   name=f"psg{s}_{n}"))
                        for n in range(2):
                            nsl = bass.ts(n, D)
                            for kc in range(KC):
                                nc.tensor.matmul(ps_g[n][:], hT[:, kc, :],
                                                 whh_sb[:, kc, nsl],
                                                 start=(kc == 0), stop=False)
                            nc.tensor.matmul(ps_g[n][:],
                                             idsel[:, erow:erow + BL],
                                             emb_pre[:, s // 16, nsl],
                                             start=False, stop=False)

                        # 2..5) e = tanh(enc_proj + dec_proj); scores = wf^T e
                        # last add chunk runs on GpSimd so DVE only does 3
                        ps_sc = ps_sc_pool.tile([1, KC, 512], F32, tag="sc",
                                                name=f"pssc{s}")
                        for c in range(KC):
                            sT = sTp.tile([128, BL, P], F16, tag="sT",
                                          name=f"sT{s}_{c}")
                            eng = nc.gpsimd if c == KC - 1 else nc.vector
                            eng.tensor_tensor(
                                sT[:],
                                enc_projT[:, c, :].rearrange("p (b q) -> p b q", b=BL),
                                decT[:, c, :, None].broadcast_to([128, BL, P]), Add)
                            eT = eTp.tile([128, BP], F16, tag="eT",
                                          name=f"eT{s}_{c}")
                            nc.scalar.activation(eT[:], sT[:].rearrange("p b q -> p (b q)"),
                                                 Tanh)
                            for n in range(KC):
                                nc.tensor.matmul(ps_sc[:, n, 0:BP // KC],
                                                 wf_sb[:, c, :],
                                                 eT[:, bass.ts(n, BP // KC)],
                                                 start=(c == 0), stop=(c == KC - 1))

                        # 6) PSUM -> SBUF row, DMA-reshape to [BL, P]
                        sc_row = scrp.tile([1, KC, BP // KC], F32, tag="scr",
                                           name=f"scrow{s}")
                        nc.vector.tensor_copy(sc_row[:], ps_sc[:, :, 0:BP // KC])
                        nc.sync.dma_start(scores_buf[:, 0:P],
                                          sc_row[:].rearrange("o n q -> o (n q)"))

                        # 7..9) softmax, in place (|scores| < ~2, no max-shift)
                        sumexp = smp.tile([BL, 1], F32, tag="sm", name=f"sumexp{s}")
                        nc.scalar.activation(scores_buf[:, 0:P], scores_buf[:, 0:P],
                                             Exp, accum_out=sumexp[:])
                        rec = smp.tile([BL, 1], F32, tag="sm", name=f"rec{s}")
                        nc.vector.reciprocal(rec[:], sumexp[:])
                        nc.vector.tensor_scalar_mul(scores_buf[:, 0:P],
                                                    scores_buf[:, 0:P], rec[:])
                        alpha = scores_buf

                        # 10) transpose alpha (P padded to 256), one-copy
                        # scatter into block-diagonal alphaD [128, 2*BL, BL]
                        ps_tr_a = ps_small.tile([128, 2, BL], F32, tag="small",
                                                name=f"pstra{s}")
                        for j in range(2):
                            nc.tensor.transpose(ps_tr_a[:, j, :],
                                                alpha[:, 128 * j:128 * (j + 1)],
                                                ident[:])
                        src_a = ps_tr_a[:]
                        src_diag = bass.AP(
                            tensor=src_a.tensor, offset=src_a.offset,
                            ap=[list(src_a.ap[0]), [1, BL], [BL, 2]])
                        nc.vector.tensor_copy(alphaD_diag, src_diag)

                        # 11) ctx[b,:] = sum_p alpha[b,p] feats[b,p,:]
                        ps_ctx = ps_cf.tile([BL, ENC], F32, tag="cf",
                                            name=f"psctx{s}")
                        for k in range(2 * BL):
                            nc.tensor.matmul(ps_ctx[:], alphaD[:, k, :],
                                             feats16[:, k, :],
                                             start=(k == 0), stop=(k == 2 * BL - 1))

                        # 13) ctx2 = (1 + tanh(zb/2)) * ctx
                        ctx2 = t2k.tile([BL, ENC], F32, tag="t2k", name=f"ctx2{s}")
                        nc.vector.scalar_tensor_tensor(ctx2[:], taub[:], 1.0,
                                                       ps_ctx[:], op0=Add, op1=Mult)

                        # 16) ctx2T [128, KC, BL]
                        ps_tr_c = ps_small.tile([128, KC, BL], F32, tag="small",
                                                name=f"pstrc{s}")
                        for c in range(KC):
                            nc.tensor.transpose(ps_tr_c[:, c, :],
                                                ctx2[:, bass.ts(c, 128)], ident[:])
                        ctx2T = tinyp.tile([128, KC, BL], BF16, tag="tiny",
                                           name=f"ctx2T{s}")
                        nc.scalar.activation(ctx2T[:], ps_tr_c[:], Ident)

                        # 17) finish gates; n=2,3 do h-part + emb here too
                        gacts = []  # tau_i, tau_f, gt, tau_o
                        for n in range(4):
                            nsl = bass.ts(n, D)
                            if n >= 2:
                                for kc in range(KC):
                                    nc.tensor.matmul(ps_g[n][:], hT[:, kc, :],
                                                     whh_sb[:, kc, nsl],
                                                     start=(kc == 0), stop=False)
                                nc.tensor.matmul(ps_g[n][:],
                                                 idsel[:, erow:erow + BL],
                                                 emb_pre[:, s // 16, nsl],
                                                 start=False, stop=False)
                            for kc in range(KC):
                                nc.tensor.matmul(ps_g[n][:], ctx2T[:, kc, :],
                                                 wih2_sb[:, kc, nsl],
                                                 start=False, stop=(kc == KC - 1))
                            ga = t2k.tile([BL, D], F32, tag="ga", name=f"ga{s}_{n}")
                            nc.scalar.activation(ga[:], ps_g[n][:], Tanh,
                                                 scale=(1.0 if n == 2 else 0.5))
                            gacts.append(ga)
                        tau_i, tau_f, gt, tau_o = gacts

                        # 22..27) LSTM cell in tanh form
                        t_a = t2k.tile([BL, D], F32, tag="t2k", name=f"ta{s}")
                        nc.vector.scalar_tensor_tensor(t_a[:], tau_f[:], 1.0,
                                                       c_prev[:], op0=Add, op1=Mult)
                        t_b = t2k.tile([BL, D], F32, tag="t2k", name=f"tb{s}")
                        nc.vector.scalar_tensor_tensor(t_b[:], tau_i[:], 1.0,
                                                       gt[:], op0=Add, op1=Mult)
                        s2 = t2k.tile([BL, D], F32, tag="t2k", name=f"s2{s}")
                        nc.vector.tensor_add(s2[:], t_a[:], t_b[:])
                        nc.vector.tensor_scalar_mul(c_next[:], s2[:], 0.5)
                        tc2 = t2k.tile([BL, D], F32, tag="t2k", name=f"tc2{s}")
                        nc.scalar.activation(tc2[:], s2[:], Tanh, scale=0.5)
                        h2 = t2k.tile([BL, D], F32, tag="t2k", name=f"h2{s}")
                        nc.vector.scalar_tensor_tensor(h2[:], tau_o[:], 1.0, tc2[:],
                                                       op0=Add, op1=Mult)

                        # 28) h2 -> H_all / H16 slot s+1
                        ps_tr_h = ps_small.tile([128, KC, BL], F32, tag="small",
                                                name=f"pstrh{s}")
                        for c in range(KC):
                            nc.tensor.transpose(ps_tr_h[:, c, :],
                                                h2[:, bass.ts(c, 128)], ident[:])
                        nc.vector.tensor_copy(H_all[:, :, s + 1, :], ps_tr_h[:])
                        nc.scalar.activation(H16[:, :, s + 1, :], ps_tr_h[:], Ident)

                        # interleaved fc pass 1 (t=1..16) once its H is final
                        if s >= FC_S0:
                            for _ in range(FC_PER_STEP):
                                if fc_next < len(V_CHUNKS):
                                    fc_chunk(fc_next, 1, 13, fcw1, "fcw1",
                                             ps_sc_pool, "sc")
                                    fc_next += 1

                # ---------------- fc pass 2 (t=14..29) ----------------
                with tc.tile_pool(name="ps_fc2", bufs=4,
                                  space="PSUM") as ps_fc2:
                    for ci in range(len(V_CHUNKS)):
                        fc_chunk(ci, 14, 16, fcw2, "fcw2", ps_fc2, "fc2")

    nc.compile()
    return nc


def _prep_core_inputs(inputs, k):
    """Host-side marshalling for core k (samples 8k..8k+8)."""
    f32 = np.float32
    bs = slice(BL * k, BL * (k + 1))
    feats = np.ascontiguousarray(inputs["encoder_feats"][bs]).astype(f32)
    pooled = np.ascontiguousarray(inputs["encoder_pooled"][bs]).astype(f32)
    caps = np.asarray(inputs["captions"][bs])

    import ml_dtypes

    d = {}
    ft = feats.transpose(2, 0, 1).reshape(KC, 128, BP)
    d["featsT"] = np.ascontiguousarray(ft.transpose(1, 0, 2)).astype(
        ml_dtypes.bfloat16)
    fp = np.zeros((128, 2 * BL, ENC), f32)
    for b in range(BL):
        fp[0:128, 2 * b] = feats[b, 0:128]
        fp[0:P - 128, 2 * b + 1] = feats[b, 128:P]
    d["feats16"] = fp.astype(ml_dtypes.bfloat16)
    d["pooledT"] = np.ascontiguousarray(
        pooled.T.reshape(KC, 128, BL).transpose(1, 0, 2))
    emb = np.asarray(inputs["emb_table"], f32)[caps[:, :S]]      # (8, 29, 512)
    et = np.zeros((128, KC, 32, BL), f32)
    g = emb.transpose(2, 1, 0).reshape(KC, 128, S, BL)
    et[:, :, 0:S, :] = g.transpose(1, 0, 2, 3)
    d["embT"] = et.astype(ml_dtypes.bfloat16)
    return d


def _prep_shared_inputs(inputs):
    f32 = np.float32
    import ml_dtypes
    bf16 = ml_dtypes.bfloat16

    def rows(w):  # (512, N) -> [128, 4, N]
        return np.ascontiguousarray(
            np.asarray(w, f32).reshape(KC, 128, -1).transpose(1, 0, 2))

    d = {}
    d["Wd16"] = rows(0.5 * np.asarray(inputs["Wd_att"], f32)).astype(bf16)
    d["wf16"] = rows(inputs["wf_att"]).astype(np.float16)
    d["Wih2"] = rows(0.5 * np.asarray(inputs["W_ih"][E:], f32)).astype(bf16)
    d["Whh"] = rows(0.5 * np.asarray(inputs["W_hh"], f32))
    d["WihE"] = rows(inputs["W_ih"][:E]).astype(bf16)
    d["Wbeta"] = rows(0.5 * np.asarray(inputs["W_beta"], f32)).astype(bf16)
    d["Winih"] = rows(inputs["W_init_h"])
    d["Winic"] = rows(inputs["W_init_c"])
    d["We"] = rows(inputs["We_att"]).astype(bf16)
    d["Wfc16"] = rows(0.5 * np.asarray(inputs["W_fc"], f32)).astype(bf16)
    d["bihh"] = (np.asarray(inputs["b_ih"], f32)
                 + np.asarray(inputs["b_hh"], f32)).reshape(1, -1)
    d["binih"] = np.asarray(inputs["b_init_h"], f32).reshape(1, -1)
    d["binic"] = np.asarray(inputs["b_init_c"], f32).reshape(1, -1)
    d["bd_til"] = np.ascontiguousarray(
        np.asarray(inputs["bd_att"], f32).reshape(KC, 128).T)
    d["be_til"] = np.ascontiguousarray(
        np.asarray(inputs["be_att"], f32).reshape(KC, 128).T)
    d["ones128"] = np.ones((1, 128), f32)
    return d


_NC_CACHE = {}


def _get_program(with_biases=False):
    if with_biases not in _NC_CACHE:
        _NC_CACHE[with_biases] = build_program(with_biases)
    return _NC_CACHE[with_biases]


def run_on_device(inputs, trace=False, **kw):
    with_biases = bool(np.any(np.asarray(inputs["b_beta"], np.float32)))
    nc = _get_program(with_biases)
    shared = _prep_shared_inputs(inputs)
    if with_biases:
        shared["bbeta"] = np.asarray(inputs["b_beta"], np.float32).reshape(1, -1)
    in_maps = []
    for k in range(NCORES):
        m = dict(shared)
        m.update(_prep_core_inputs(inputs, k))
        in_maps.append(m)
    return run_bass_kernel_spmd(nc, in_maps, list(range(NCORES)), trace=trace, **kw)


def kernel(**inputs) -> np.ndarray:
    res = run_on_device(inputs)
    parts = [res.results[k]["out_logits"] for k in range(NCORES)]
    out = np.concatenate(parts, axis=0)
    b_fc = np.asarray(inputs["b_fc"], np.float32).reshape(1, 1, V)
    out[:, 1:, :] += b_fc
    return out


# revision 18
# speedup vs baseline: 1.2152x; 1.0111x over previous
"""Trainium2 Bass kernel for an LSTM decoder with additive attention + large
vocab projection (nn_DecoderWithAttention).

Strategy: 8-way data parallel over batch (8 samples per core), zero
collectives. Recurrent state h is kept feature-major [D, B] and scaled by 2
(h' = 2h) so every sigmoid can be computed as sigma(x) = (1 + tanh(x/2)) / 2
on the ACT engine -- keeping all scalar-engine ops inside the single
"exp_and_others" activation table set. The compensating 0.5 factors are
folded into W_d/W_beta/W_hh/W_fc/W_ih2 on the host (exact power-of-two
scale).

v2 changes vs v1 (1.90 ms):
  - fc phase split into two bf16 passes over H (t=1..16 and t=17..29); the
    first pass is emitted interleaved into steps 16..28 so its matmuls/DMAs
    hide in the recurrent loop's idle time. W_fc and the fc copy of h are
    bf16 (~0.4% rounding on logits only).
  - PSUM rebudgeted (8 banks: small 1 / scores 4 / ctx+fc 1 / beta+gates 2)
    so the beta matmul and ~half of the gates h-part matmuls prefill at step
    start, overlapping the attention phase.
  - Attention tensors (enc_projT / decT / sT / eT / wf) in fp16: halves
    SBUF + lets the scores matmul stream fp16.
  - alpha scatter into the block-diagonal tile is one strided tensor_copy.
  - dec_proj uses bf16 Wd x bf16 h' (1 cyc/row vs fp32's 4).

Per-core per-step dataflow (s = 0..28):
  dec_projT [A,B]  = (0.5 Wd)^T @ h'          (PE, bf16)
  beta/gates h-part matmuls prefill           (PE, during attention)
  eT[A,(B,P)]      = tanh(enc_projT + dec_projT bcast)   (DVE add, ACT tanh)
  scores[1,(B,P)]  = wf^T @ eT                (PE, fp16 streaming)
  alpha            = exp(scores)/sum          (ACT exp + fused accum)
  ctx[B,ENC]       = alpha @ feats            (PE, block-diag 16-K accum)
  ctx2             = (1 + tanh(zb/2)) * ctx   (= 2 sigmoid(zb) ctx)
  gates[B,4D]      = emb_pre[s] + ctx2@(W_ih2/2) + h'@(W_hh/2)
  LSTM cell in tanh form; h' = (1+tanh(o/2)) * tanh(c2)
Output row t=0 stays zero (buffer pre-zeroed + explicit zero DMA).
"""

import os
import sys

for _p in ("/opt/trn_rl_repo", os.path.expanduser("~/.axon_site/_ro/trn_rl_repo")):
    if os.path.isdir(_p) and _p not in sys.path:
        sys.path.insert(0, _p)

import numpy as np

import concourse.bass as bass
import concourse.tile as tile
from concourse import bacc, mybir
from concourse.bass_utils import run_bass_kernel_spmd
from concourse.masks import make_identity

F32 = mybir.dt.float32
F32R = mybir.dt.float32r
F16 = mybir.dt.float16
BF16 = mybir.dt.bfloat16

B, P, T = 64, 196, 30
E, D, A, ENC, V = 512, 512, 512, 512, 30000
NCORES = 8
BL = B // NCORES          # 8 samples per core
S = T - 1                 # 29 recurrent steps
BP = BL * P               # 1568
KC = 4                    # 128-row chunks per 512 feature dim
VCHUNK = 512
V_CHUNKS = [(i * VCHUNK, min(VCHUNK, V - i * VCHUNK))
            for i in range((V + VCHUNK - 1) // VCHUNK)]
# fc pass 1 (t=1..13, 104 rows) is interleaved into steps FC_S0..28.
FC_S0 = 13
FC_PER_STEP = 4

Tanh = mybir.ActivationFunctionType.Tanh
Exp = mybir.ActivationFunctionType.Exp
Ident = mybir.ActivationFunctionType.Identity
Add = mybir.AluOpType.add
Mult = mybir.AluOpType.mult


def r(ap):
    return ap.bitcast(F32R)


def build_program(with_biases=False):
    nc = bacc.Bacc(
        "TRN2",
        target_bir_lowering=False,
        debug=False,
        enable_asserts=False,
        num_devices=NCORES,
    )

    def din(name, shape, dt=F32):
        return nc.dram_tensor(name, list(shape), dt, kind="ExternalInput").ap()

    featsT_d = din("featsT", (128, KC, BP), BF16)          # [p,c,b*196+q] = feats[b,q,128c+p]
    feats16_d = din("feats16", (128, 2 * BL, ENC), BF16)   # (b,p) rows, P padded to 256
    pooledT_d = din("pooledT", (128, KC, BL), F32R)
    embT_d = din("embT", (128, KC, 32, BL), BF16)          # [p,c,t,b], t<29 used
    Wd16_d = din("Wd16", (128, KC, A), BF16)               # 0.5*Wd_att rows, bf16
    wf_d = din("wf16", (128, KC, 1), F16)
    Wih2_d = din("Wih2", (128, KC, 4 * D), BF16)           # 0.5*W_ih[512:] rows
    Whh_d = din("Whh", (128, KC, 4 * D), F32R)             # 0.5*W_hh rows
    WihE_d = din("WihE", (128, KC, 4 * D), BF16)           # W_ih[:512] rows
    Wbeta_d = din("Wbeta", (128, KC, ENC), BF16)           # 0.5*W_beta rows
    Winih_d = din("Winih", (128, KC, D), F32R)
    Winic_d = din("Winic", (128, KC, D), F32R)
    We_d = din("We", (128, KC, A), BF16)                   # We_att rows
    Wfc16_d = din("Wfc16", (128, KC, V), BF16)             # 0.5*W_fc rows, bf16
    bihh_d = din("bihh", (1, 4 * D), F32R)                 # b_ih + b_hh
    binih_d = din("binih", (1, D), F32R)
    binic_d = din("binic", (1, D), F32R)
    bd_d = din("bd_til", (128, KC))                        # bd_att as [p, c]
    be_d = din("be_til", (128, KC))
    ones_d = din("ones128", (1, 128), F32R)
    if with_biases:
        bbeta_d = din("bbeta", (1, ENC), F32R)

    out_d = nc.dram_tensor("out_logits", [BL, T, V], F32, kind="ExternalOutput").ap()
    outT = out_d.rearrange("b t v -> t b v")

    with tile.TileContext(nc) as tc:
        with tc.tile_pool(name="const", bufs=1) as const:
            wd_sb = const.tile([128, KC, A], BF16)
            wf_sb = const.tile([128, KC, 1], F16)
            wih2_sb = const.tile([128, KC, 4 * D], BF16)
            whh_sb = const.tile([128, KC, 4 * D], F32R)
            wbeta_sb = const.tile([128, KC, ENC], BF16)
            enc_projT = const.tile([128, KC, BP], F16)
            emb_pre = const.tile([128, 2, 4 * D], F32R)
            H_all = const.tile([128, KC, T, BL], F32R)   # slot t: h' after t steps
            H16 = const.tile([128, KC, T, BL], BF16)     # bf16 copy for fc + dec
            bd_sb = const.tile([128, KC], F32)
            be_sb = const.tile([128, KC], F32)
            ident = const.tile([BL, BL], F32)
            ident128 = const.tile([128, 128], F32)
            idsel = const.tile([128, 128], F32R)
            c_state = [const.tile([BL, D], F32, tag=f"cstate{i}", name=f"c_state{i}")
                       for i in range(2)]
            if with_biases:
                ones_lp = const.tile([1, 128], F32R)
                bbeta_sb = const.tile([1, ENC], F32R)
                nc.sync.dma_start(ones_lp[:], ones_d)
                nc.sync.dma_start(bbeta_sb[:], bbeta_d)

            nc.sync.dma_start(wd_sb[:], Wd16_d)
            nc.sync.dma_start(wf_sb[:], wf_d)
            nc.sync.dma_start(wih2_sb[:], Wih2_d)
            nc.sync.dma_start(whh_sb[:], Whh_d)
            nc.sync.dma_start(wbeta_sb[:], Wbeta_d)
            nc.sync.dma_start(bd_sb[:], bd_d)
            nc.sync.dma_start(be_sb[:], be_d)
            make_identity(nc, ident[:])
            make_identity(nc, ident128[:])
            nc.vector.tensor_copy(idsel[:], ident128[:])

            # ---------------- setup phase ----------------
            with tc.tile_pool(name="setup", bufs=1) as setup, \
                 tc.tile_pool(name="setup2", bufs=2) as setup2, \
                 tc.tile_pool(name="setup_ps", bufs=2, space="PSUM") as setup_ps:

                pooledT_sb = setup.tile([128, KC, BL], F32R)
                ones_sb = setup.tile([1, 128], F32R)
                bihh_sb = setup.tile([1, 4 * D], F32R)
                binih_sb = setup.tile([1, D], F32R)
                binic_sb = setup.tile([1, D], F32R)
                winih_sb = setup.tile([128, KC, D], F32R)
                winic_sb = setup.tile([128, KC, D], F32R)
                nc.sync.dma_start(pooledT_sb[:], pooledT_d)
                nc.sync.dma_start(ones_sb[:], ones_d)
                nc.sync.dma_start(bihh_sb[:], bihh_d)
                nc.sync.dma_start(binih_sb[:], binih_d)
                nc.sync.dma_start(binic_sb[:], binic_d)
                nc.sync.dma_start(winih_sb[:], Winih_d)
                nc.sync.dma_start(winic_sb[:], Winic_d)

                # h0/c0 (B-major): lhsT = pooledT chunks, rhs = W_init rows
                for which in range(2):
                    w_sb = winih_sb if which == 0 else winic_sb
                    b_row = binih_sb if which == 0 else binic_sb
                    ps = setup_ps.tile([BL, D], F32, tag="init_ps")
                    for kc in range(KC):
                        nc.tensor.matmul(ps[:], pooledT_sb[:, kc, :],
                                         w_sb[:, kc, :], start=(kc == 0), stop=False)
                    nc.tensor.matmul(ps[:], ones_sb[0:1, 0:BL], b_row[0:1, :],
                                     start=False, stop=True)
                    if which == 0:
                        h0 = setup.tile([BL, D], F32)
                        nc.scalar.activation(h0[:], ps[:], Tanh)
                        h0x2 = setup.tile([BL, D], F32)
                        nc.vector.tensor_scalar_mul(h0x2[:], h0[:], 2.0)
                        trps = setup_ps.tile([128, KC, BL], F32, tag="tr_ps")
                        for c in range(KC):
                            nc.tensor.transpose(trps[:, c, :],
                                                h0x2[:, c * 128:(c + 1) * 128],
                                                ident[:])
                        nc.vector.tensor_copy(H_all[:, :, 0, :], trps[:])
                        nc.scalar.activation(H16[:, :, 0, :], trps[:], Ident)
                    else:
                        nc.scalar.activation(c_state[0][:], ps[:], Tanh)

                # enc_projT = We^T @ featsT + be  (A-major, fp16 out)
                we_sb = setup.tile([128, KC, A], BF16)
                nc.sync.dma_start(we_sb[:], We_d)
                for n in range(KC):
                    nsl = bass.ts(n, BP // KC)  # 392 cols
                    ft_stage = setup2.tile([128, KC, BP // KC], BF16, tag="ftst")
                    nc.sync.dma_start(ft_stage[:], featsT_d[:, :, nsl])
                    for c in range(KC):
                        ps = setup_ps.tile([128, BP // KC], F32, tag="enc_ps")
                        for kc in range(KC):
                            nc.tensor.matmul(ps[:], we_sb[:, kc, bass.ts(c, 128)],
                                             ft_stage[:, kc, :],
                                             start=(kc == 0), stop=(kc == KC - 1))
                        nc.scalar.activation(enc_projT[:, c, nsl], ps[:], Ident,
                                             bias=be_sb[:, c:c + 1])

                # emb_pre = embT^T @ W_ih[:512] + (b_ih + b_hh), rows (t, b)
                nc.vector.memset(emb_pre[:].bitcast(F32), 0.0)
                embT_sb = setup.tile([128, KC, 32, BL], BF16)
                nc.sync.dma_start(embT_sb[:], embT_d)
                for n in range(4):
                    nsl = bass.ts(n, 512)
                    wst = setup2.tile([128, KC, 512], BF16, tag="wihE")
                    nc.sync.dma_start(wst[:], WihE_d[:, :, nsl])
                    for mt, (t0, nt) in enumerate([(0, 16), (16, 13)]):
                        rows = nt * BL
                        ps = setup_ps.tile([128, 512], F32, tag="emb_ps")
                        for kc in range(KC):
                            nc.tensor.matmul(ps[0:rows, :],
                                             embT_sb[:, kc, t0:t0 + nt, :],
                                             wst[:, kc, :], start=(kc == 0),
                                             stop=False)
                        nc.tensor.matmul(ps[0:rows, :], ones_sb[0:1, 0:rows],
                                         bihh_sb[0:1, nsl], start=False, stop=True)
                        nc.vector.tensor_copy(emb_pre[0:rows, mt, nsl], ps[0:rows, :])

            # ------------- recurrent loop + interleaved fc pass 1 -------------
            with tc.tile_pool(name="fcw1", bufs=3) as fcw1, \
                 tc.tile_pool(name="fcw2", bufs=7) as fcw2, \
                 tc.tile_pool(name="fco", bufs=3) as fco:

                def fc_chunk(ci, t0, nt, wpool, wtag, pspool, pstag,
                             wst=None):
                    v0, vn = V_CHUNKS[ci]
                    rows = nt * BL
                    if wst is None:
                        wst = wpool.tile([128, KC, VCHUNK], BF16, tag=wtag,
                                         name=f"fw{wtag}{ci}")
                        nc.sync.dma_start(wst[:, :, 0:vn],
                                          Wfc16_d[:, :, v0:v0 + vn])
                    ps = pspool.tile([128, VCHUNK], F32, tag=pstag,
                                     name=f"psfc{t0}_{ci}")
                    for kc in range(KC):
                        nc.tensor.matmul(ps[0:rows, 0:vn],
                                         H16[:, kc, t0:t0 + nt, :],
                                         wst[:, kc, 0:vn],
                                         start=(kc == 0), stop=(kc == KC - 1))
                    ost = fco.tile([128, VCHUNK], F32, tag="fco",
                                   name=f"fo{t0}_{ci}")
                    nc.vector.tensor_copy(ost[0:rows, 0:vn], ps[0:rows, 0:vn])
                    nc.sync.dma_start(outT[t0:t0 + nt, :, v0:v0 + vn],
                                      ost[0:rows, 0:vn])

                # output row t=0 is defined to be zeros
                zt = fco.tile([1, 400], F32, tag="zt")
                nc.vector.memset(zt[:], 0.0)
                zsrc = zt[0:1, 0:400]
                zsrc = bass.AP(tensor=zsrc.tensor, offset=zsrc.offset,
                               ap=[list(zsrc.ap[0]), [0, BL * V // 400], [1, 400]])
                nc.sync.dma_start(out_d[:, 0, :], zsrc)

                with tc.tile_pool(name="lper", bufs=1) as lper, \
                     tc.tile_pool(name="sT", bufs=2) as sTp, \
                     tc.tile_pool(name="eT", bufs=2) as eTp, \
                     tc.tile_pool(name="scr", bufs=2) as scrp, \
                     tc.tile_pool(name="t2k", bufs=4) as t2k, \
                     tc.tile_pool(name="tiny", bufs=3) as tinyp, \
                     tc.tile_pool(name="sm", bufs=2) as smp, \
                     tc.tile_pool(name="ps_small", bufs=1, space="PSUM") as ps_small, \
                     tc.tile_pool(name="ps_sc", bufs=1, space="PSUM") as ps_sc_pool, \
                     tc.tile_pool(name="ps_bg", bufs=3, space="PSUM") as ps_bg:

                    feats16 = lper.tile([128, 2 * BL, ENC], BF16)
                    nc.sync.dma_start(feats16[:], feats16_d)
                    alphaD = lper.tile([128, 2 * BL, BL], BF16)
                    nc.vector.memset(alphaD[:], 0.0)
                    scores_buf = lper.tile([BL, 256], F32)
                    nc.vector.memset(scores_buf[:, P:256], 0.0)
                    # strided view of alphaD for the one-copy block-diag
                    # scatter: dst[b, j] = alphaD[:, 2b+j, b]
                    aD0 = alphaD[:]
                    alphaD_diag = bass.AP(
                        tensor=aD0.tensor, offset=aD0.offset,
                        ap=[list(aD0.ap[0]), [2 * BL + 1, BL], [BL, 2]])

                    fc_next = 0
                    fc2_pre = []
                    for s in range(S):
                        hT = H_all[:, :, s, :]
                        c_prev = c_state[s % 2]
                        c_next = c_state[(s + 1) % 2]

                        # 1) dec_projT [128, KC, BL] (bf16 Wd x bf16 h')
                        ps_dec = ps_small.tile([128, KC, BL], F32, tag="small",
                                               name=f"psdec{s}")
                        for m in range(KC):
                            for kc in range(KC):
                                nc.tensor.matmul(ps_dec[:, m, :],
                                                 wd_sb[:, kc, bass.ts(m, 128)],
                                                 H16[:, kc, s, :],
                                                 start=(kc == 0),
                                                 stop=(kc == KC - 1))
                        decT = tinyp.tile([128, KC, BL], F16, tag="tiny",
                                          name=f"decT{s}")
                        for c in range(KC):
                            nc.scalar.activation(decT[:, c, :], ps_dec[:, c, :],
                                                 Ident, bias=bd_sb[:, c:c + 1])

                        # beta gate preact (h-only): prefills during attention
                        ps_b = ps_small.tile([BL, ENC], F32, tag="small",
                                             name=f"psb{s}")
                        for kc in range(KC):
                            nc.tensor.matmul(ps_b[:], H16[:, kc, s, :],
                                             wbeta_sb[:, kc, :],
                                             start=(kc == 0),
                                             stop=(not with_biases and kc == KC - 1))
                        if with_biases:
                            nc.tensor.matmul(ps_b[:], ones_lp[0:1, 0:BL],
                                             bbeta_sb[0:1, :], start=False, stop=True)
                        taub = t2k.tile([BL, ENC], F32, tag="t2k", name=f"taub{s}")
                        nc.scalar.activation(taub[:], ps_b[:], Tanh, scale=0.5)

                        # gates: h-part + emb selector prefill for n=0,1
                        erow = (s % 16) * BL
                        ps_g = []
                        for n in range(4):
                            ps_g.append(ps_bg.tile([BL, D], F32, tag="bg",
                                                   name=f"psg{s}_{n}"))
                        for n in range(3):
                            nsl = bass.ts(n, D)
                            for kc in range(KC):
                                nc.tensor.matmul(ps_g[n][:], hT[:, kc, :],
                                                 whh_sb[:, kc, nsl],
                                                 start=(kc == 0), stop=False)
                            nc.tensor.matmul(ps_g[n][:],
                                             idsel[:, erow:erow + BL],
                                             emb_pre[:, s // 16, nsl],
                                             start=False, stop=False)

                        # 2..5) e = tanh(enc_proj + dec_proj); scores = wf^T e
                        # last add chunk runs on GpSimd so DVE only does 3
                        ps_sc = ps_sc_pool.tile([1, KC, 512], F32, tag="sc",
                                                name=f"pssc{s}")
                        for c in range(KC):
                            sT = sTp.tile([128, BL, P], F16, tag="sT",
                                          name=f"sT{s}_{c}")
                            eng = nc.gpsimd if c == KC - 1 else nc.vector
                            eng.tensor_tensor(
                                sT[:],
                                enc_projT[:, c, :].rearrange("p (b q) -> p b q", b=BL),
                                decT[:, c, :, None].broadcast_to([128, BL, P]), Add)
                            eT = eTp.tile([128, BP], F16, tag="eT",
                                          name=f"eT{s}_{c}")
                            nc.scalar.activation(eT[:], sT[:].rearrange("p b q -> p (b q)"),
                                                 Tanh)
                            for n in range(KC):
                                nc.tensor.matmul(ps_sc[:, n, 0:BP // KC],
                                                 wf_sb[:, c, :],
                                                 eT[:, bass.ts(n, BP // KC)],
                                                 start=(c == 0), stop=(c == KC - 1))

                        # 6) PSUM -> SBUF row, DMA-reshape to [BL, P]
                        sc_row = scrp.tile([1, KC, BP // KC], F32, tag="scr",
                                           name=f"scrow{s}")
                        nc.vector.tensor_copy(sc_row[:], ps_sc[:, :, 0:BP // KC])
                        nc.sync.dma_start(scores_buf[:, 0:P],
                                          sc_row[:].rearrange("o n q -> o (n q)"))

                        # 7..9) softmax, in place (|scores| < ~2, no max-shift)
                        sumexp = smp.tile([BL, 1], F32, tag="sm", name=f"sumexp{s}")
                        nc.scalar.activation(scores_buf[:, 0:P], scores_buf[:, 0:P],
                                             Exp, accum_out=sumexp[:])
                        rec = smp.tile([BL, 1], F32, tag="sm", name=f"rec{s}")
                        nc.vector.reciprocal(rec[:], sumexp[:])
                        nc.vector.tensor_scalar_mul(scores_buf[:, 0:P],
                                                    scores_buf[:, 0:P], rec[:])
                        alpha = scores_buf

                        # 10) transpose alpha (P padded to 256), one-copy
                        # scatter into block-diagonal alphaD [128, 2*BL, BL]
                        ps_tr_a = ps_small.tile([128, 2, BL], F32, tag="small",
                                                name=f"pstra{s}")
                        for j in range(2):
                            nc.tensor.transpose(ps_tr_a[:, j, :],
                                                alpha[:, 128 * j:128 * (j + 1)],
                                                ident[:])
                        src_a = ps_tr_a[:]
                        src_diag = bass.AP(
                            tensor=src_a.tensor, offset=src_a.offset,
                            ap=[list(src_a.ap[0]), [1, BL], [BL, 2]])
                        nc.vector.tensor_copy(alphaD_diag, src_diag)

                        # 11) ctx[b,:] = sum_p alpha[b,p] feats[b,p,:]
                        ps_ctx = ps_small.tile([BL, ENC], F32, tag="small",
                                               name=f"psctx{s}")
                        for k in range(2 * BL):
                            nc.tensor.matmul(ps_ctx[:], alphaD[:, k, :],
                                             feats16[:, k, :],
                                             start=(k == 0), stop=(k == 2 * BL - 1))

                        # 13) ctx2 = (1 + tanh(zb/2)) * ctx
                        ctx2 = t2k.tile([BL, ENC], F32, tag="t2k", name=f"ctx2{s}")
                        nc.vector.scalar_tensor_tensor(ctx2[:], taub[:], 1.0,
                                                       ps_ctx[:], op0=Add, op1=Mult)

                        # 16) ctx2T [128, KC, BL]
                        ps_tr_c = ps_small.tile([128, KC, BL], F32, tag="small",
                                                name=f"pstrc{s}")
                        for c in range(KC):
                            nc.tensor.transpose(ps_tr_c[:, c, :],
                                                ctx2[:, bass.ts(c, 128)], ident[:])
                        ctx2T = tinyp.tile([128, KC, BL], BF16, tag="tiny",
                                           name=f"ctx2T{s}")
                        nc.scalar.activation(ctx2T[:], ps_tr_c[:], Ident)

                        # 17) finish gates; n=2,3 do h-part + emb here too
                        gacts = []  # tau_i, tau_f, gt, tau_o
                        for n in range(4):
                            nsl = bass.ts(n, D)
                            if n >= 3:
                                for kc in range(KC):
                                    nc.tensor.matmul(ps_g[n][:], hT[:, kc, :],
                                                     whh_sb[:, kc, nsl],
                                                     start=(kc == 0), stop=False)
                                nc.tensor.matmul(ps_g[n][:],
                                                 idsel[:, erow:erow + BL],
                                                 emb_pre[:, s // 16, nsl],
                                                 start=False, stop=False)
                            for kc in range(KC):
                                nc.tensor.matmul(ps_g[n][:], ctx2T[:, kc, :],
                                                 wih2_sb[:, kc, nsl],
                                                 start=False, stop=(kc == KC - 1))
                            ga = t2k.tile([BL, D], F32, tag="ga", name=f"ga{s}_{n}")
                            nc.scalar.activation(ga[:], ps_g[n][:], Tanh,
                                                 scale=(1.0 if n == 2 else 0.5))
                            gacts.append(ga)
                        tau_i, tau_f, gt, tau_o = gacts

                        # 22..27) LSTM cell in tanh form
                        t_a = t2k.tile([BL, D], F32, tag="t2k", name=f"ta{s}")
                        nc.vector.scalar_tensor_tensor(t_a[:], tau_f[:], 1.0,
                                                       c_prev[:], op0=Add, op1=Mult)
                        t_b = t2k.tile([BL, D], F32, tag="t2k", name=f"tb{s}")
                        nc.vector.scalar_tensor_tensor(t_b[:], tau_i[:], 1.0,
                                                       gt[:], op0=Add, op1=Mult)
                        s2 = t2k.tile([BL, D], F32, tag="t2k", name=f"s2{s}")
                        nc.vector.tensor_add(s2[:], t_a[:], t_b[:])
                        nc.vector.tensor_scalar_mul(c_next[:], s2[:], 0.5)
                        tc2 = t2k.tile([BL, D], F32, tag="t2k", name=f"tc2{s}")
                        nc.scalar.activation(tc2[:], s2[:], Tanh, scale=0.5)
                        h2 = t2k.tile([BL, D], F32, tag="t2k", name=f"h2{s}")
                        nc.vector.scalar_tensor_tensor(h2[:], tau_o[:], 1.0, tc2[:],
                                                       op0=Add, op1=Mult)

                        # 28) h2 -> H_all / H16 slot s+1
                        ps_tr_h = ps_small.tile([128, KC, BL], F32, tag="small",
                                                name=f"pstrh{s}")
                        for c in range(KC):
                            nc.tensor.transpose(ps_tr_h[:, c, :],
                                                h2[:, bass.ts(c, 128)], ident[:])
                        nc.vector.tensor_copy(H_all[:, :, s + 1, :], ps_tr_h[:])
                        nc.scalar.activation(H16[:, :, s + 1, :], ps_tr_h[:], Ident)

                        # pre-stage the first pass-2 weight chunks so the
                        # tail starts with weights already resident
                        if s == 5:
                            for ci in range(7):
                                v0, vn = V_CHUNKS[ci]
                                w = fcw2.tile([128, KC, VCHUNK], BF16,
                                              tag="fcw2", name=f"fw2pre{ci}")
                                nc.sync.dma_start(w[:, :, 0:vn],
                                                  Wfc16_d[:, :, v0:v0 + vn])
                                fc2_pre.append(w)

                        # interleaved fc pass 1 (t=1..13) once its H is final
                        if s >= FC_S0:
                            for _ in range(FC_PER_STEP):
                                if fc_next < len(V_CHUNKS):
                                    fc_chunk(fc_next, 1, 13, fcw1, "fcw1",
                                             ps_sc_pool, "sc")
                                    fc_next += 1

                # ---------------- fc pass 2 (t=14..29) ----------------
                with tc.tile_pool(name="ps_fc2", bufs=4,
                                  space="PSUM") as ps_fc2:
                    for ci in range(len(V_CHUNKS)):
                        fc_chunk(ci, 14, 16, fcw2, "fcw2", ps_fc2, "fc2",
                                 wst=(fc2_pre[ci] if ci < len(fc2_pre)
                                      else None))

    nc.compile()
    return nc


def _prep_core_inputs(inputs, k):
    """Host-side marshalling for core k (samples 8k..8k+8)."""
    f32 = np.float32
    bs = slice(BL * k, BL * (k + 1))
    feats = np.ascontiguousarray(inputs["encoder_feats"][bs]).astype(f32)
    pooled = np.ascontiguousarray(inputs["encoder_pooled"][bs]).astype(f32)
    caps = np.asarray(inputs["captions"][bs])

    import ml_dtypes

    d = {}
    ft = feats.transpose(2, 0, 1).reshape(KC, 128, BP)
    d["featsT"] = np.ascontiguousarray(ft.transpose(1, 0, 2)).astype(
        ml_dtypes.bfloat16)
    fp = np.zeros((128, 2 * BL, ENC), f32)
    for b in range(BL):
        fp[0:128, 2 * b] = feats[b, 0:128]
        fp[0:P - 128, 2 * b + 1] = feats[b, 128:P]
    d["feats16"] = fp.astype(ml_dtypes.bfloat16)
    d["pooledT"] = np.ascontiguousarray(
        pooled.T.reshape(KC, 128, BL).transpose(1, 0, 2))
    emb = np.asarray(inputs["emb_table"], f32)[caps[:, :S]]      # (8, 29, 512)
    et = np.zeros((128, KC, 32, BL), f32)
    g = emb.transpose(2, 1, 0).reshape(KC, 128, S, BL)
    et[:, :, 0:S, :] = g.transpose(1, 0, 2, 3)
    d["embT"] = et.astype(ml_dtypes.bfloat16)
    return d


def _prep_shared_inputs(inputs):
    f32 = np.float32
    import ml_dtypes
    bf16 = ml_dtypes.bfloat16

    def rows(w):  # (512, N) -> [128, 4, N]
        return np.ascontiguousarray(
            np.asarray(w, f32).reshape(KC, 128, -1).transpose(1, 0, 2))

    d = {}
    d["Wd16"] = rows(0.5 * np.asarray(inputs["Wd_att"], f32)).astype(bf16)
    d["wf16"] = rows(inputs["wf_att"]).astype(np.float16)
    d["Wih2"] = rows(0.5 * np.asarray(inputs["W_ih"][E:], f32)).astype(bf16)
    d["Whh"] = rows(0.5 * np.asarray(inputs["W_hh"], f32))
    d["WihE"] = rows(inputs["W_ih"][:E]).astype(bf16)
    d["Wbeta"] = rows(0.5 * np.asarray(inputs["W_beta"], f32)).astype(bf16)
    d["Winih"] = rows(inputs["W_init_h"])
    d["Winic"] = rows(inputs["W_init_c"])
    d["We"] = rows(inputs["We_att"]).astype(bf16)
    d["Wfc16"] = rows(0.5 * np.asarray(inputs["W_fc"], f32)).astype(bf16)
    d["bihh"] = (np.asarray(inputs["b_ih"], f32)
                 + np.asarray(inputs["b_hh"], f32)).reshape(1, -1)
    d["binih"] = np.asarray(inputs["b_init_h"], f32).reshape(1, -1)
    d["binic"] = np.asarray(inputs["b_init_c"], f32).reshape(1, -1)
    d["bd_til"] = np.ascontiguousarray(
        np.asarray(inputs["bd_att"], f32).reshape(KC, 128).T)
    d["be_til"] = np.ascontiguousarray(
        np.asarray(inputs["be_att"], f32).reshape(KC, 128).T)
    d["ones128"] = np.ones((1, 128), f32)
    return d


_NC_CACHE = {}


def _get_program(with_biases=False):
    if with_biases not in _NC_CACHE:
        _NC_CACHE[with_biases] = build_program(with_biases)
    return _NC_CACHE[with_biases]


def run_on_device(inputs, trace=False, **kw):
    with_biases = bool(np.any(np.asarray(inputs["b_beta"], np.float32)))
    nc = _get_program(with_biases)
    shared = _prep_shared_inputs(inputs)
    if with_biases:
        shared["bbeta"] = np.asarray(inputs["b_beta"], np.float32).reshape(1, -1)
    in_maps = []
    for k in range(NCORES):
        m = dict(shared)
        m.update(_prep_core_inputs(inputs, k))
        in_maps.append(m)
    return run_bass_kernel_spmd(nc, in_maps, list(range(NCORES)), trace=trace, **kw)


def kernel(**inputs) -> np.ndarray:
    res = run_on_device(inputs)
    parts = [res.results[k]["out_logits"] for k in range(NCORES)]
    out = np.concatenate(parts, axis=0)
    b_fc = np.asarray(inputs["b_fc"], np.float32).reshape(1, 1, V)
    out[:, 1:, :] += b_fc
    return out
